# revision 1
# baseline (speedup 1.0000x reference)
"""Autoformer encoder (B=32, L=1024, D=256, 3 layers) on 8 TRN2 NeuronCores.

Data-parallel over batch (4 batches/core). All matmuls in f32r (full PE
rate, ~1.6e-4 rel err), fp32 residual stream and vector path.

AutoCorrelation without FFT: the lag-correlation
    C[tau] = (1/D) sum_l <q[:, l+tau], k[:, l]>
is computed as F[p, u] = sum_i sum_d k[d, 128i+p] * q2[d, 128i+u]
(PSUM-accumulated matmuls, q2 time-doubled), so that
C[tau] = sum_p F[p, p+tau]. The 128-row shear is done by bouncing F
through DRAM with row stride 1153 and reading back the strided view
[[1154, 128], [1, 1024]]; the partition sum is a ones-vector matmul.
Top-6 lags via vector.max/max_index.

The delay-rolled weighted sum of V uses register-dynamic slices into a
time-doubled V buffer. Each dynamic-AP instruction permanently consumes
~2 registers of the executing engine's 49 (no caching in this build), so
the 72 total gather slots are spread: 23 on ACT (scaled copy), 23 on DVE
(scalar_tensor_tensor FMA), 22 on Pool (FMA), 4 on PE (scaled-identity
matmul with dynamic rhs, PSUM-accumulated).
"""

import contextlib
import numpy as np
import ml_dtypes

import concourse.bass as bass
import concourse.mybir as mybir
from concourse import tile
from concourse.tile import TileContext
from concourse.tile_rust import add_dep_helper
from concourse.vector_clock import ScopedClock
from concourse.bass_utils import run_bass_kernel_spmd

F32 = mybir.dt.float32
F32R = mybir.dt.float32r
BF16 = mybir.dt.bfloat16
U32 = mybir.dt.uint32
AF = mybir.ActivationFunctionType
AX = mybir.AxisListType
ALU = mybir.AluOpType
ET = mybir.EngineType

B, L, C_IN = 32, 1024, 21
D, DFF, NL = 256, 1024, 3
TOPK = 6
NCORES = 8
BL = B // NCORES  # batches per core

HW = 1153  # F bounce row stride (1152 data + 1 pad)
FSH_SZ = 127 * HW + 1152


# ---------------------------------------------------------------- walrus fix
def _patched_drain_and_barrier(self, tick_clock, wait_clock):
    nc = self.nc
    drain_inst = nc.sync.drain()
    wait_clock.add_sem_waits(
        drain_inst.ins, ScopedClock({None: tick_clock.global_clock})
    )
    si = drain_inst.ins.sync_info
    if si is not None and len(si.on_wait) > 1:
        extra = list(si.on_wait[1:])
        del si.on_wait[1:]
        for w in extra:
            n = nc.sync.nop()
            n.ins.sync_info = mybir.SyncInfo(on_update=[], on_wait=[w])
    nc.all_engine_barrier()
    assert self.sems is not None
    popped = nc._tile_sem_poison_stack.pop()
    assert popped is self._sem_poison
    nc.clear_and_free_semaphores(list(self.sems.allocated().values()))
    nc.all_engine_barrier()


tile.TileContext._drain_and_barrier = _patched_drain_and_barrier

_wsctr = [0]


def _split_control_waits(nc):
    """This walrus build allows only ONE sync wait per instruction;
    hoist extras onto NoOps just before, same engine."""
    for fn in nc.m.functions:
        for bb in fn.blocks:
            out = []
            changed = False
            for inst in bb.instructions:
                si = getattr(inst, "sync_info", None)
                if si is not None and len(si.on_wait) > 1:
                    extra = list(si.on_wait[1:])
                    del si.on_wait[1:]
                    for w in extra:
                        _wsctr[0] += 1
                        n = mybir.InstNoOp(
                            name=f"I-waitsplit-{_wsctr[0]}", ins=[], outs=[]
                        )
                        n.engine = inst.engine
                        n.sync_info = mybir.SyncInfo(on_update=[], on_wait=[w])
                        out.append(n)
                        changed = True
                out.append(inst)
            if changed:
                bb.instructions[:] = out


def r(ap):
    return ap


def dep(a, b):
    add_dep_helper(a.ins, b.ins, sync=False, reason="gather order")


# ---------------------------------------------------------------- builder
def build_nc():
    nc = bass.Bass()
    P = lambda name, shape, dt=F32: nc.declare_dram_parameter(
        name, shape, dt, isOutput=False
    )
    xemb = P("xemb", [BL, 63, L], F32R)  # host im2col of token conv input
    tokw = P("tokw", [63, D], F32R)  # lhsT for token conv
    wq = P("wq", [NL, D, D], F32R)  # lhsT (= W.T) per layer
    wk = P("wk", [NL, D, D], F32R)
    wv = P("wv", [NL, D, D], F32R)
    wo = P("wo", [NL, D, D], F32R)
    wc1 = P("wc1", [NL, D, DFF], F32R)  # lhsT
    wc2 = P("wc2", [NL, DFF, D], F32R)  # lhsT
    nwp = P("nw", [D, 1])
    nbp = P("nb", [D, 1])
    pw = P("pw", [D, 3, L], BF16)  # proj_w as [d, class, l], bf16
    pb = P("pb", [1, 3])
    onescol = P("onescol", [128, 1], F32R)
    onescolf = P("onescolf", [128, 1])
    onesrow = P("onesrow", [1, 128])
    ident = P("ident", [128, 128])
    out = nc.declare_dram_parameter("out", [BL, 3], F32, isOutput=True)
    import os
    KDBG = bool(os.environ.get("KDBG"))
    dbg = {}
    if KDBG:
        for nm_, shp, dt_ in [
            ("dbg_x0", [2, 128, L], F32),
            ("dbg_k", [128, L], F32),
            ("dbg_q2", [128, 4096], F32),
            ("dbg_f", [128, 1152], F32),
            ("dbg_h", [128, L], F32),
            ("dbg_c", [1, L], F32),
            ("dbg_ix", [1, 8], U32),
            ("dbg_wb", [128, 8], F32),
            ("dbg_a", [128, 2048], F32),
            ("dbg_s", [2, 128, L + 4], F32),
            ("dbg_x1", [2, 128, L + 4], F32),
            ("dbg_xo", [2, 128, L], F32),
        ]:
            dbg[nm_] = nc.declare_dram_parameter(nm_, shp, dt_, isOutput=True)

    fsh = nc.dram_tensor("fsh", [BL * NL, FSH_SZ], F32R)

    with TileContext(nc) as tc:
        ctx = contextlib.ExitStack()
        with ctx:
            wp = ctx.enter_context(tc.tile_pool(name="weights", bufs=1))
            res = ctx.enter_context(tc.tile_pool(name="res", bufs=6))
            scr = ctx.enter_context(tc.tile_pool(name="scr", bufs=8))
            big = ctx.enter_context(tc.tile_pool(name="big4k", bufs=3))
            gat = ctx.enter_context(tc.tile_pool(name="gat", bufs=3))
            sp = ctx.enter_context(tc.tile_pool(name="small", bufs=4))
            ps = ctx.enter_context(tc.tile_pool(name="psum", bufs=3, space="PSUM"))
            ps2p = ctx.enter_context(
                tc.tile_pool(name="psumB", bufs=2, space="PSUM")
            )

            _names = [0]

            def _nm(pfx):
                _names[0] += 1
                return f"{pfx}{_names[0]}"

            def rtile():
                return res.tile([128, L + 4], F32R, tag="res", name=_nm("rt"))

            def stile(fr=1152, dt=F32, p=128):
                return scr.tile([p, fr], dt, tag="scr", name=_nm("st"))

            # ---- load weights to SBUF once
            tokw_sb = wp.tile([63, D], F32R, tag="tokw")
            nc.sync.dma_start(out=tokw_sb[:], in_=tokw[:])
            ones_sb = wp.tile([128, 1], F32R, tag="ones")
            ones2_sb = wp.tile([128, 1], F32, tag="ones2")
            nc.sync.dma_start(out=ones_sb[:], in_=onescol[:])
            nc.sync.dma_start(out=ones2_sb[:], in_=onescolf[:])
            onesr_sb = wp.tile([1, 128], F32, tag="onesr")
            nc.sync.dma_start(out=onesr_sb[:], in_=onesrow[:])
            id_sb = wp.tile([128, 128], F32, tag="id")
            nc.sync.dma_start(out=id_sb[:], in_=ident[:])
            nw_sb = wp.tile([128, 2], F32, tag="nw")  # col t = tile t
            nb_sb = wp.tile([128, 2], F32, tag="nb")
            for t in range(2):
                nc.sync.dma_start(
                    out=nw_sb[:, t : t + 1], in_=nwp[t * 128 : (t + 1) * 128, :]
                )
                nc.sync.dma_start(
                    out=nb_sb[:, t : t + 1], in_=nbp[t * 128 : (t + 1) * 128, :]
                )
            pb_sb = wp.tile([1, 3], F32, tag="pb")
            nc.sync.dma_start(out=pb_sb[:], in_=pb[:])

            # layer weights streamed per (b, l), double-buffered
            ws = ctx.enter_context(tc.tile_pool(name="wstream", bufs=2))

            def lload(name, src, l, kt, cols):
                tl = ws.tile(
                    [128, cols], F32R, tag=f"{name}k{kt}", name=_nm(f"{name}{l}")
                )
                nc.sync.dma_start(out=tl[:], in_=src[l, kt * 128 : (kt + 1) * 128, :])
                return tl
            pw_sb = [None, None]
            for t in range(2):
                pw_sb[t] = wp.tile([128, 3 * L], BF16, tag=f"pw{t}", name=f"pw{t}")
                nc.sync.dma_start(
                    out=pw_sb[t][:].rearrange("p (c l) -> p c l", c=3),
                    in_=pw[t * 128 : (t + 1) * 128, :, :],
                )

            # persistent per-engine delay registers + snapped values
            engs = {
                "ACT": nc.engines[ET.Activation],
                "DVE": nc.engines[ET.DVE],
                "POOL": nc.engines[ET.Pool],
                "PE": nc.engines[ET.PE],
            }
            dreg = {k: e.alloc_register(f"dly_{k}") for k, e in engs.items()}
            dval = {
                k: nc.snap(rg, donate=True, min_val=0, max_val=1023)
                for k, rg in dreg.items()
            }

            def proj(dst_fn, w_sb_l, src_aps):
                """dst[mt][chunk] <- sum_kt w[kt].T @ src[kt][:, chunk]."""
                for mt in range(2):
                    for ch in range(2):
                        p5 = ps2p.tile([128, 512], F32, tag="mm512", name=_nm("p5"))
                        for kt in range(2):
                            nc.tensor.matmul(
                                p5[:],
                                r(w_sb_l[kt][:, mt * 128 : (mt + 1) * 128]),
                                r(src_aps[kt][:, ch * 512 : (ch + 1) * 512]),
                                start=(kt == 0),
                                stop=(kt == 1),
                            )
                        dst_fn(mt, ch, p5)

            def batch_program(b):
                # ---- token embedding: x[d, l], 2 tiles, data in [0, L)
                xe_sb = stile(fr=L, p=63, dt=F32R)
                nc.sync.dma_start(out=xe_sb[:], in_=xemb[b, :, :])
                x_sb = [rtile() for _ in range(2)]
                for mt in range(2):
                    for ch in range(2):
                        p5 = ps2p.tile([128, 512], F32, tag="mm512", name=_nm("p5"))
                        nc.tensor.matmul(
                            p5[:],
                            r(tokw_sb[:, mt * 128 : (mt + 1) * 128]),
                            r(xe_sb[:, ch * 512 : (ch + 1) * 512]),
                            start=True,
                            stop=True,
                        )
                        nc.vector.tensor_copy(
                            x_sb[mt][:, ch * 512 : (ch + 1) * 512], p5[:]
                        )

                if KDBG and b == 0:
                    for t in range(2):
                        nc.sync.dma_start(
                            out=dbg["dbg_x0"][t], in_=x_sb[t][:, 0:L].bitcast(F32)
                        )

                for l in range(NL):
                    last_bl = (b == BL - 1) and (l == NL - 1)
                    tap = KDBG and b == 0 and l == 0
                    wq_l = [lload("wq", wq, l, t, D) for t in range(2)]
                    wk_l = [lload("wk", wk, l, t, D) for t in range(2)]
                    wv_l = [lload("wv", wv, l, t, D) for t in range(2)]
                    wo_l = [lload("wo", wo, l, t, D) for t in range(2)]
                    wc1_l = [lload("wc1", wc1, l, t, DFF) for t in range(2)]
                    wc2_l = [lload("wc2", wc2, l, t, D) for t in range(8)]
                    # ---- Q (doubled, stacked kt: col 2048*kt + u), K, V (same)
                    q2_sb = big.tile([128, 4096], F32R, tag="big4k", name=_nm("q2"))
                    v4_sb = big.tile([128, 4096], F32R, tag="big4k", name=_nm("v4"))
                    k_sb = [stile(dt=F32R) for _ in range(2)]

                    def dbl_out(dst):
                        def f(mt, ch, p5):
                            base = 2048 * mt + ch * 512
                            nc.vector.tensor_copy(dst[:, base : base + 512], p5[:])
                            nc.scalar.copy(dst[:, base + 1024 : base + 1536], p5[:])

                        return f

                    def k_out(mt, ch, p5):
                        nc.scalar.copy(
                            k_sb[mt][:, ch * 512 : (ch + 1) * 512], p5[:]
                        )

                    xin = [x_sb[t][:, 0:L] for t in range(2)]
                    proj(dbl_out(q2_sb), wq_l, xin)
                    proj(k_out, wk_l, xin)
                    proj(dbl_out(v4_sb), wv_l, xin)

                    if tap:
                        nc.sync.dma_start(
                            out=dbg["dbg_k"][:], in_=k_sb[0][:, 0:L].bitcast(F32)
                        )
                        nc.sync.dma_start(
                            out=dbg["dbg_q2"][:], in_=q2_sb[:].bitcast(F32)
                        )

                    # ---- F[p, u] = sum_i sum_d k[d,128i+p] q2[d,128i+u]
                    # F in two PSUM tiles so "big" slots stay 2 banks and F
                    # can overlap the FFN's ps2 accumulators. Each 384-wide
                    # chunk is bank-aligned (a matmul output may not cross a
                    # 512-f32 PSUM bank).
                    fps_a = ps.tile([128, 1024], F32, tag="big", name=_nm("fpsa"))
                    fps_b = ps2p.tile([128, 512], F32, tag="mm512", name=_nm("fpsb"))
                    for ch in range(3):  # 3 x 384
                        dstp = (
                            fps_a[:, ch * 512 : ch * 512 + 384]
                            if ch < 2
                            else fps_b[:, 0:384]
                        )
                        for i in range(8):
                            for kt in range(2):
                                base = 2048 * kt + i * 128 + ch * 384
                                nc.tensor.matmul(
                                    dstp,
                                    r(k_sb[kt][:, i * 128 : (i + 1) * 128]),
                                    r(q2_sb[:, base : base + 384]),
                                    start=((i, kt) == (0, 0)),
                                    stop=((i, kt) == (7, 1)),
                                )
                    # bounce through DRAM with the shear stride
                    f_sb = stile(dt=F32R)
                    nc.vector.tensor_copy(
                        f_sb[:, 0:768].rearrange("p (c u) -> p c u", c=2),
                        fps_a[:].rearrange("p (c u) -> p c u", c=2)[:, :, 0:384],
                    )
                    nc.vector.tensor_copy(f_sb[:, 768:1152], fps_b[:, 0:384])
                    frow = fsh[b * NL + l, :]
                    wview = bass.AP(frow.tensor, frow.offset, [[HW, 128], [1, 1152]])
                    fwr = nc.sync.dma_start(out=wview, in_=f_sb[:, 0:1152])
                    hview = bass.AP(
                        frow.tensor, frow.offset, [[HW + 1, 128], [1, 1024]]
                    )
                    h_sb = stile(dt=F32R)
                    hrd = nc.sync.dma_start(out=h_sb[:, 0:1024], in_=hview)
                    add_dep_helper(
                        hrd.ins, fwr.ins, sync=True, reason="hankel read after write"
                    )
                    yield
                    if tap:
                        nc.sync.dma_start(
                            out=dbg["dbg_f"][:], in_=f_sb[:, 0:1152].bitcast(F32)
                        )
                        nc.sync.dma_start(
                            out=dbg["dbg_h"][:], in_=h_sb[:, 0:1024].bitcast(F32)
                        )

                    # ---- C[tau] = (1/256) * sum_p H[p, tau]; top-6; softmax
                    c_sb = stile()
                    for ch in range(2):
                        cp = ps2p.tile([1, 512], F32, tag="mm512", name=_nm("cp"))
                        nc.tensor.matmul(
                            cp[:],
                            r(ones_sb[:]),
                            r(h_sb[:, ch * 512 : (ch + 1) * 512]),
                            start=True,
                            stop=True,
                        )
                        nc.scalar.activation(
                            c_sb[:1, ch * 512 : (ch + 1) * 512],
                            cp[:],
                            AF.Copy,
                            scale=1.0 / D,
                        )
                    mx = sp.tile([1, 8], F32, tag="mx", name=_nm("mx"))
                    ix = sp.tile([1, 8], U32, tag="ix", name=_nm("ix"))
                    nc.vector.max(out=mx[:], in_=c_sb[:1, 0:1024])
                    nc.vector.max_index(
                        out=ix[:], in_max=mx[:], in_values=c_sb[:1, 0:1024]
                    )
                    negmax = sp.tile([1, 1], F32, tag="negmax", name=_nm("ng"))
                    nc.vector.tensor_scalar_mul(negmax[:], mx[:1, 0:1], -1.0)
                    ex = sp.tile([1, 8], F32, tag="ex", name=_nm("ex"))
                    nc.scalar.activation(
                        ex[:1, 0:TOPK], mx[:1, 0:TOPK], AF.Exp, bias=negmax[:1, 0:1]
                    )
                    esum = sp.tile([1, 1], F32, tag="esum", name=_nm("es"))
                    nc.vector.reduce_sum(esum[:], ex[:1, 0:TOPK], axis=AX.X)
                    rinv = sp.tile([1, 1], F32, tag="rinv", name=_nm("ri"))
                    nc.vector.reciprocal(rinv[:], esum[:])
                    wts = sp.tile([1, 8], F32, tag="wts", name=_nm("wt"))
                    nc.vector.tensor_scalar_mul(
                        wts[:1, 0:TOPK], ex[:1, 0:TOPK], rinv[:1, 0:1]
                    )
                    # broadcast weights to all 128 partitions
                    psw = ps2p.tile([128, TOPK], F32, tag="mm512", name=_nm("pw_"))
                    nc.tensor.matmul(
                        psw[:], onesr_sb[:], wts[:1, 0:TOPK], start=True, stop=True
                    )
                    wb = sp.tile([128, TOPK], F32, tag="wb", name=_nm("wb"))
                    nc.vector.tensor_copy(wb[:], psw[:])
                    if tap:
                        nc.sync.dma_start(out=dbg["dbg_c"][:], in_=c_sb[:1, 0:L])
                        nc.sync.dma_start(out=dbg["dbg_ix"][:], in_=ix[:])
                        nc.sync.dma_start(
                            out=dbg["dbg_wb"][:, 0:TOPK], in_=wb[:]
                        )

                    # ---- a[:, 1024*t + u] = sum_i w_i V[t][:, (u+d_i) % L]
                    a_sb = gat.tile([128, 2048], F32R, tag="gat", name=_nm("a"))
                    tq_sb = gat.tile([128, 2048], F32R, tag="gat", name=_nm("tq"))
                    pq_sb = gat.tile([128, 2048], F32R, tag="gat", name=_nm("pq"))
                    v4r = v4_sb[:].rearrange("p (b u) -> p b u", b=2)
                    a3 = a_sb[:].rearrange("p (b u) -> p b u", b=2)
                    tq3 = tq_sb[:].rearrange("p (b u) -> p b u", b=2)
                    pq3 = pq_sb[:].rearrange("p (b u) -> p b u", b=2)

                    def ld(ekey, i):
                        return engs[ekey].reg_load(dreg[ekey], ix[:1, i : i + 1])

                    def act_copy(i, dst3):
                        return nc.scalar.activation(
                            dst3,
                            v4r[:, :, bass.ds(dval["ACT"], 1024)],
                            AF.Copy,
                            scale=wb[:, i : i + 1],
                        )

                    def fma(ekey, i):
                        eng = nc.vector if ekey == "DVE" else nc.gpsimd
                        return eng.scalar_tensor_tensor(
                            a3,
                            v4r[:, :, bass.ds(dval[ekey], 1024)],
                            wb[:, i : i + 1],
                            a3,
                            op0=ALU.mult,
                            op1=ALU.add,
                        )

                    if not last_bl:
                        l0 = ld("ACT", 0)
                        o0 = act_copy(0, a3)
                        dep(o0, l0)
                        l1 = ld("ACT", 1)
                        dep(l1, o0)
                        o1 = act_copy(1, tq3)
                        dep(o1, l1)
                        l2 = ld("DVE", 2)
                        o2 = fma("DVE", 2)
                        dep(o2, l2)
                        l3 = ld("DVE", 3)
                        dep(l3, o2)
                        o3_ = fma("DVE", 3)
                        dep(o3_, l3)
                        # Pool: tensor_tensor mult with broadcast weight
                        l4 = ld("POOL", 4)
                        o4 = nc.gpsimd.tensor_mul(
                            pq3,
                            v4r[:, :, bass.ds(dval["POOL"], 1024)],
                            wb[:, 4:5].to_broadcast([128, 2, 1024]),
                        )
                        dep(o4, l4)
                        ad4 = nc.vector.tensor_add(a_sb[:], a_sb[:], pq_sb[:])
                        l5 = ld("POOL", 5)
                        dep(l5, o4)
                        o5 = nc.gpsimd.tensor_mul(
                            pq3,
                            v4r[:, :, bass.ds(dval["POOL"], 1024)],
                            wb[:, 5:6].to_broadcast([128, 2, 1024]),
                        )
                        dep(o5, l5)
                        nc.vector.tensor_add(a_sb[:], a_sb[:], pq_sb[:])
                        nc.vector.tensor_add(a_sb[:], a_sb[:], tq_sb[:])
                    else:
                        # last (b, l): ACT slot 0, DVE slot 1, PE slots 2..5
                        l0 = ld("ACT", 0)
                        o0 = act_copy(0, a3)
                        dep(o0, l0)
                        l1 = ld("DVE", 1)
                        o1 = fma("DVE", 1)
                        dep(o1, l1)
                        pe = engs["PE"]
                        wds = []
                        for i in range(2, 6):
                            wd = stile(fr=128, dt=F32R)
                            nc.vector.tensor_scalar(
                                wd[:, 0:128],
                                id_sb[:],
                                wb[:, i : i + 1],
                                None,
                                op0=ALU.mult,
                            )
                            wds.append(wd)
                        pgs = []
                        prev = None
                        for t in range(2):
                            for c in range(2):
                                pg = ps2p.tile(
                                    [128, 512], F32, tag="mm512", name=_nm("pg")
                                )
                                for ii, i in enumerate(range(2, 6)):
                                    lp = pe.reg_load(dreg["PE"], ix[:1, i : i + 1])
                                    if prev is not None:
                                        dep(lp, prev)
                                    al = pe.reg_alu(
                                        dreg["PE"],
                                        dreg["PE"],
                                        2048 * t + 512 * c,
                                        ALU.add,
                                    )
                                    dep(al, lp)
                                    mm = nc.tensor.matmul(
                                        pg[:],
                                        r(wds[ii][:, 0:128]),
                                        r(v4_sb[:, bass.ds(dval["PE"], 512)]),
                                        start=(ii == 0),
                                        stop=(ii == 3),
                                    )
                                    dep(mm, al)
                                    prev = mm
                                pgs.append((t, c, pg))
                        for t, c, pg in pgs:
                            base = 1024 * t + 512 * c
                            nc.vector.tensor_add(
                                a_sb[:, base : base + 512],
                                a_sb[:, base : base + 512],
                                pg[:],
                            )

                    if tap:
                        nc.sync.dma_start(
                            out=dbg["dbg_a"][:], in_=a_sb[:].bitcast(F32)
                        )

                    yield

                    # ---- O-projection; s = x + a into padded tile (data at 2)
                    s_sb = [rtile() for _ in range(2)]

                    def o_out(mt, ch, p5):
                        nc.vector.tensor_add(
                            s_sb[mt][:, 2 + ch * 512 : 2 + (ch + 1) * 512],
                            x_sb[mt][:, ch * 512 : (ch + 1) * 512],
                            p5[:],
                        )

                    proj(
                        o_out,
                        wo_l,
                        [a_sb[:, 1024 * t : 1024 * (t + 1)] for t in range(2)],
                    )

                    # ---- series_decomp (dst may alias src data cols)
                    def decomp(src_pad, dst, dst_off):
                        # src_pad: [128, 1028] with data in cols [2, 1026)
                        sv = src_pad[:, 2:1026]
                        nc.vector.tensor_copy(
                            src_pad[:, 0:2], src_pad[:, 2:3].to_broadcast([128, 2])
                        )
                        nc.vector.tensor_copy(
                            src_pad[:, 1026:1028],
                            src_pad[:, 1025:1026].to_broadcast([128, 2]),
                        )
                        a2 = stile()
                        nc.gpsimd.tensor_add(
                            a2[:, 0:1027], src_pad[:, 0:1027], src_pad[:, 1:1028]
                        )
                        a4 = stile()
                        nc.vector.tensor_add(
                            a4[:, 0:1025], a2[:, 0:1025], a2[:, 2:1027]
                        )
                        m5 = stile()
                        nc.vector.tensor_add(
                            m5[:, 0:1024], a4[:, 0:1024], src_pad[:, 4:1028]
                        )
                        # dst = (m5 * -0.2) + sv, fused
                        nc.vector.scalar_tensor_tensor(
                            dst[:, dst_off : dst_off + 1024],
                            m5[:, 0:1024],
                            -0.2,
                            sv,
                            op0=ALU.mult,
                            op1=ALU.add,
                        )

                    if tap:
                        for t in range(2):
                            nc.sync.dma_start(
                                out=dbg["dbg_s"][t], in_=s_sb[t][:].bitcast(F32)
                            )

                    # x1 = decomp(s) in place (x1 aliases s_sb data cols)
                    for t in range(2):
                        decomp(s_sb[t], s_sb[t], dst_off=2)
                    x1_sb = s_sb
                    if tap:
                        for t in range(2):
                            nc.sync.dma_start(
                                out=dbg["dbg_x1"][t], in_=x1_sb[t][:].bitcast(F32)
                            )

                    # ---- FFN: y = gelu(c1 @ x1); s2 = x1 + c2 @ y (in place)
                    x1v = [x1_sb[t][:, 2:1026] for t in range(2)]
                    ps2 = [
                        ps.tile([128, 1024], F32, tag="big", name=_nm("ps2"))
                        for _ in range(2)
                    ]
                    for ft in range(8):
                        y_sb = stile(dt=F32R)
                        for ch in range(2):
                            p5 = ps2p.tile([128, 512], F32, tag="mm512", name=_nm("p5"))
                            for kt in range(2):
                                nc.tensor.matmul(
                                    p5[:],
                                    r(wc1_l[kt][:, ft * 128 : (ft + 1) * 128]),
                                    r(x1v[kt][:, ch * 512 : (ch + 1) * 512]),
                                    start=(kt == 0),
                                    stop=(kt == 1),
                                )
                            nc.scalar.activation(
                                y_sb[:, ch * 512 : (ch + 1) * 512], p5[:], AF.Gelu
                            )
                        for mt in range(2):
                            for ch in range(2):
                                nc.tensor.matmul(
                                    ps2[mt][:, ch * 512 : (ch + 1) * 512],
                                    r(wc2_l[ft][:, mt * 128 : (mt + 1) * 128]),
                                    r(y_sb[:, ch * 512 : (ch + 1) * 512]),
                                    start=(ft == 0),
                                    stop=(ft == 7),
                                )
                    for mt in range(2):
                        for ch in range(2):
                            nc.vector.tensor_add(
                                x1v[mt][:, ch * 512 : (ch + 1) * 512],
                                x1v[mt][:, ch * 512 : (ch + 1) * 512],
                                ps2[mt][:, ch * 512 : (ch + 1) * 512],
                            )
                    for t in range(2):
                        decomp(x1_sb[t], x_sb[t], dst_off=0)
                    yield
                    if tap:
                        for t in range(2):
                            nc.sync.dma_start(
                                out=dbg["dbg_xo"][t], in_=x_sb[t][:, 0:L].bitcast(F32)
                            )

                # ---- my_layernorm + gelu + head
                xv = [x_sb[t][:, 0:L] for t in range(2)]
                xsq = [stile(dt=F32R) for _ in range(2)]
                for t in range(2):
                    nc.scalar.activation(xsq[t][:, 0:L], xv[t], AF.Square)
                mu = stile()
                ex2 = stile()
                for ch in range(2):
                    cs = ps2p.tile([1, 512], F32, tag="mm512", name=_nm("cs"))
                    for kt in range(2):
                        nc.tensor.matmul(
                            cs[:],
                            r(ones_sb[:]),
                            r(xv[kt][:, ch * 512 : (ch + 1) * 512]),
                            start=(kt == 0),
                            stop=(kt == 1),
                        )
                    nc.scalar.activation(
                        mu[:1, ch * 512 : (ch + 1) * 512], cs[:], AF.Copy, scale=1.0 / D
                    )
                    cq = ps2p.tile([1, 512], F32, tag="mm512", name=_nm("cq"))
                    for kt in range(2):
                        nc.tensor.matmul(
                            cq[:],
                            r(ones_sb[:]),
                            r(xsq[kt][:, ch * 512 : (ch + 1) * 512]),
                            start=(kt == 0),
                            stop=(kt == 1),
                        )
                    nc.scalar.activation(
                        ex2[:1, ch * 512 : (ch + 1) * 512],
                        cq[:],
                        AF.Copy,
                        scale=1.0 / D,
                    )
                epsb = sp.tile([1, 1], F32, tag="epsb", name=_nm("ep"))
                nc.vector.memset(epsb[:], 1e-5)
                musq = stile()
                nc.vector.tensor_mul(musq[:1, 0:1024], mu[:1, 0:1024], mu[:1, 0:1024])
                nc.vector.tensor_sub(
                    ex2[:1, 0:1024], ex2[:1, 0:1024], musq[:1, 0:1024]
                )
                nc.scalar.activation(
                    ex2[:1, 0:1024], ex2[:1, 0:1024], AF.Sqrt, bias=epsb[:1, 0:1]
                )
                nc.vector.reciprocal(ex2[:1, 0:1024], ex2[:1, 0:1024])  # rstd
                # broadcast mu, rstd to 128 partitions
                mub = stile()
                rstdb = stile()
                for src, dst in ((mu, mub), (ex2, rstdb)):
                    for ch in range(2):
                        pbd = ps2p.tile([128, 512], F32, tag="mm512", name=_nm("pb_"))
                        nc.tensor.matmul(
                            pbd[:],
                            onesr_sb[:],
                            src[:1, ch * 512 : (ch + 1) * 512],
                            start=True,
                            stop=True,
                        )
                        nc.vector.tensor_copy(dst[:, ch * 512 : (ch + 1) * 512], pbd[:])
                g_sb = [stile(dt=BF16) for _ in range(2)]
                for t in range(2):
                    xh = stile()
                    nc.vector.tensor_sub(xh[:, 0:L], xv[t], mub[:, 0:L])
                    nc.vector.tensor_mul(xh[:, 0:L], xh[:, 0:L], rstdb[:, 0:L])
                    nc.scalar.activation(
                        xh[:, 0:L],
                        xh[:, 0:L],
                        AF.Identity,
                        bias=nb_sb[:, t : t + 1],
                        scale=nw_sb[:, t : t + 1],
                    )
                    rowm = sp.tile([128, 1], F32, tag="rowm", name=_nm("rm"))
                    nc.vector.reduce_sum(rowm[:], xh[:, 0:L], axis=AX.X)
                    nc.vector.tensor_scalar_mul(rowm[:], rowm[:], 1.0 / L)
                    nc.vector.tensor_scalar_sub(xh[:, 0:L], xh[:, 0:L], rowm[:, 0:1])
                    nc.scalar.activation(g_sb[t][:, 0:L], xh[:, 0:L], AF.Gelu)

                # head: out[c] = sum_{t,p,l} g[t][p,l] * pw[t][p, c, l] + pb
                hsum = sp.tile([128, 8], F32, tag="hsum", name=_nm("hs"))
                for t in range(2):
                    for c in range(3):
                        hscr = stile()
                        nc.vector.tensor_mul(
                            hscr[:, 0:L],
                            g_sb[t][:, 0:L],
                            pw_sb[t][:, c * L : (c + 1) * L],
                        )
                        nc.vector.reduce_sum(
                            hsum[:, t * 3 + c : t * 3 + c + 1],
                            hscr[:, 0:L],
                            axis=AX.X,
                        )
                psh = ps2p.tile([1, 6], F32, tag="mm512", name=_nm("ph"))
                nc.tensor.matmul(
                    psh[:], ones2_sb[:], hsum[:, 0:6], start=True, stop=True
                )
                h6 = sp.tile([1, 6], F32, tag="h6", name=_nm("h6"))
                nc.vector.tensor_copy(h6[:], psh[:1, 0:6])
                o3 = sp.tile([1, 3], F32, tag="o3", name=_nm("o3"))
                nc.vector.tensor_add(o3[:], h6[:1, 0:3], h6[:1, 3:6])
                nc.vector.tensor_add(o3[:], o3[:], pb_sb[:])
                nc.sync.dma_start(out=out[b : b + 1, :], in_=o3[:])

            for pair in range(BL // 2):
                pending = [batch_program(2 * pair), batch_program(2 * pair + 1)]
                while pending:
                    for g_ in list(pending):
                        try:
                            next(g_)
                        except StopIteration:
                            pending.remove(g_)

    _split_control_waits(nc)
    return nc


# ---------------------------------------------------------------- host side
_CACHE = {}


def _get_nc():
    if "nc" not in _CACHE:
        _CACHE["nc"] = build_nc()
    return _CACHE["nc"]


def kernel(**inputs):
    x_enc = np.asarray(inputs["x_enc"], dtype=np.float32)  # (B, L, C_IN)
    token_w = np.asarray(inputs["token_w"], dtype=np.float32)
    qw = np.asarray(inputs["qw"], dtype=np.float32)
    kw = np.asarray(inputs["kw"], dtype=np.float32)
    vw = np.asarray(inputs["vw"], dtype=np.float32)
    ow = np.asarray(inputs["ow"], dtype=np.float32)
    c1w = np.asarray(inputs["c1w"], dtype=np.float32)
    c2w = np.asarray(inputs["c2w"], dtype=np.float32)
    norm_w = np.asarray(inputs["norm_w"], dtype=np.float32)
    norm_b = np.asarray(inputs["norm_b"], dtype=np.float32)
    proj_w = np.asarray(inputs["proj_w"], dtype=np.float32)
    proj_b = np.asarray(inputs["proj_b"], dtype=np.float32)

    # host-side layout marshalling (no arithmetic)
    tokw = np.ascontiguousarray(token_w.transpose(1, 2, 0).reshape(63, D))
    # xemb[b, c*3+j, l] = x_enc[b, (l+j-1) % L, c]
    xt = x_enc.transpose(0, 2, 1)  # (B, C, L)
    xemb = np.ascontiguousarray(
        np.stack([np.roll(xt, 1 - j, axis=2) for j in range(3)], axis=2).reshape(
            B, 63, L
        )
    )
    shared = {
        "tokw": tokw,
        "wq": np.ascontiguousarray(qw.transpose(0, 2, 1)),
        "wk": np.ascontiguousarray(kw.transpose(0, 2, 1)),
        "wv": np.ascontiguousarray(vw.transpose(0, 2, 1)),
        "wo": np.ascontiguousarray(ow.transpose(0, 2, 1)),
        "wc1": np.ascontiguousarray(c1w.transpose(0, 2, 1)),
        "wc2": np.ascontiguousarray(c2w.transpose(0, 2, 1)),
        "nw": norm_w.reshape(D, 1).copy(),
        "nb": norm_b.reshape(D, 1).copy(),
        "pw": np.ascontiguousarray(
            proj_w.reshape(3, L, D).transpose(2, 0, 1)
        ).astype(ml_dtypes.bfloat16),
        "pb": proj_b.reshape(1, 3).copy(),
        "onescol": np.ones((128, 1), np.float32),
        "onescolf": np.ones((128, 1), np.float32),
        "onesrow": np.ones((1, 128), np.float32),
        "ident": np.eye(128, dtype=np.float32),
    }
    in_maps = []
    for core in range(NCORES):
        m = dict(shared)
        m["xemb"] = np.ascontiguousarray(xemb[core * BL : (core + 1) * BL])
        in_maps.append(m)

    nc = _get_nc()
    res_ = run_bass_kernel_spmd(nc, in_maps, core_ids=list(range(NCORES)))
    out = np.concatenate([res_.results[i]["out"] for i in range(NCORES)], axis=0)
    return out.astype(np.float32)


if __name__ == "__main__":
    import reference

    inputs = reference.setup_inputs()
    got = kernel(**{k: np.asarray(v) for k, v in inputs.items()})
    exp = np.asarray(reference.reference(**inputs))
    rel = np.abs(got - exp).max() / np.abs(exp).max()
    print("Relative error:", rel)



# revision 16
# speedup vs baseline: 1.6382x; 1.6382x over previous
"""Autoformer encoder (B=32, L=1024, D=256, 3 layers) on 8 TRN2 NeuronCores.

Data-parallel over batch (4 batches/core), software-pipelined wavefront:
the 4 batch programs are emitted with a stage stagger (STAG) so matmul
phases of one batch overlap vector phases of another, and layer weights
are loaded once per layer (double-buffered) and shared by all batches.

Precision split (validated against the reference on host):
  - f32r: residual stream x (updated in place through all layers), q2/k,
    the lag-correlation F and C, the decomp outputs. The top-6 lag
    selection is numerically fragile; bf16/fp16 rounding anywhere ahead
    of it flips selections and blows the error to ~3e-2.
  - fp16: vo (= x @ (ow@vw).T, O-projection folded into V on host), the
    gather partial accumulators, the decomp moving-sum tree, FFN y, wc2,
    the classifier head. fp16 gets the DVE 2x/4x fast modes.

AutoCorrelation without FFT: C[tau] = sum_p F[p, p+tau] with
F[p, u] = sum_i sum_d k[d, 128i+p] * q[d, (128i+u) % L] via PSUM-
accumulated matmuls (wrapping chunks split in two, q not duplicated).
The 128-row shear is a DRAM bounce with row stride 1153. Top-6 lags via
vector.max/max_index.

The delay-rolled weighted sum of vo uses register-dynamic slices into a
time-doubled fp16 vo buffer. Each dynamic-AP instruction permanently
consumes ~2 of the executing engine's 49 registers, so the 72 gather
slots are spread: per (batch, layer) unit two DVE, two ACT, two Pool;
the final unit uses ACT 1 / DVE 1 / PE 4 (scaled-identity matmuls with
dynamic rhs).

PSUM: F 3 banks + FFN accumulator 2 banks + 3 working banks = 8.
"""

import contextlib
import numpy as np

import concourse.bass as bass
import concourse.mybir as mybir
from concourse import tile
from concourse.tile import TileContext
from concourse.tile_rust import add_dep_helper
from concourse.vector_clock import ScopedClock
from concourse.bass_utils import run_bass_kernel_spmd

F32 = mybir.dt.float32
F32R = mybir.dt.float32r
F16 = mybir.dt.float16
U32 = mybir.dt.uint32
AF = mybir.ActivationFunctionType
AX = mybir.AxisListType
ALU = mybir.AluOpType
ET = mybir.EngineType

B, L, C_IN = 32, 1024, 21
D, DFF, NL = 256, 1024, 3
TOPK = 6
NCORES = 8
BL = B // NCORES  # batches per core
SEG = 1028  # residual tile segment stride: 2 halo + 1024 + 2 halo

HW = 1153  # F bounce row stride (1152 data + 1 pad)
FSH_SZ = 127 * HW + 1152
STAG = 2  # wavefront stagger in stages between consecutive batches


# ---------------------------------------------------------------- walrus fix
def _patched_drain_and_barrier(self, tick_clock, wait_clock):
    nc = self.nc
    drain_inst = nc.sync.drain()
    wait_clock.add_sem_waits(
        drain_inst.ins, ScopedClock({None: tick_clock.global_clock})
    )
    si = drain_inst.ins.sync_info
    if si is not None and len(si.on_wait) > 1:
        extra = list(si.on_wait[1:])
        del si.on_wait[1:]
        for w in extra:
            n = nc.sync.nop()
            n.ins.sync_info = mybir.SyncInfo(on_update=[], on_wait=[w])
    nc.all_engine_barrier()
    assert self.sems is not None
    popped = nc._tile_sem_poison_stack.pop()
    assert popped is self._sem_poison
    nc.clear_and_free_semaphores(list(self.sems.allocated().values()))
    nc.all_engine_barrier()


tile.TileContext._drain_and_barrier = _patched_drain_and_barrier

_wsctr = [0]


def _split_control_waits(nc):
    """This walrus build allows only ONE sync wait per instruction;
    hoist extras onto NoOps just before, same engine."""
    for fn in nc.m.functions:
        for bb in fn.blocks:
            out = []
            changed = False
            for inst in bb.instructions:
                si = getattr(inst, "sync_info", None)
                if si is not None and len(si.on_wait) > 1:
                    extra = list(si.on_wait[1:])
                    del si.on_wait[1:]
                    for w in extra:
                        _wsctr[0] += 1
                        n = mybir.InstNoOp(
                            name=f"I-waitsplit-{_wsctr[0]}", ins=[], outs=[]
                        )
                        n.engine = inst.engine
                        n.sync_info = mybir.SyncInfo(on_update=[], on_wait=[w])
                        out.append(n)
                        changed = True
                out.append(inst)
            if changed:
                bb.instructions[:] = out


def r(ap):
    return ap


def dep(a, b):
    add_dep_helper(a.ins, b.ins, sync=False, reason="gather order")


# ---------------------------------------------------------------- builder
def build_nc():
    nc = bass.Bass()
    P = lambda name, shape, dt=F32: nc.declare_dram_parameter(
        name, shape, dt, isOutput=False
    )
    xemb = P("xemb", [BL, 63, L], F32R)  # host im2col of token conv input
    tokw = P("tokw", [63, D], F32R)  # lhsT for token conv
    wq = P("wq", [NL, D, D], F32R)  # lhsT (= W.T) per layer
    wk = P("wk", [NL, D, D], F32R)
    wvo = P("wvo", [NL, D, D], F32R)  # lhsT of (ow @ vw)
    wc1 = P("wc1", [NL, D, DFF], F32R)  # lhsT
    wc2 = P("wc2", [NL, DFF, D], F16)  # lhsT, fp16
    nwp = P("nw", [D, 1])
    nbp = P("nb", [D, 1])
    pw = P("pw", [D, 3, L], F16)  # proj_w as [d, class, l]
    pb = P("pb", [1, 3])
    onescol = P("onescol", [128, 1], F32R)
    onescolf = P("onescolf", [128, 1])
    onesrow = P("onesrow", [1, 128])
    identh = P("identh", [128, 128], F16)
    out = nc.declare_dram_parameter("out", [BL, 3], F32, isOutput=True)
    import os

    KDBG = bool(os.environ.get("KDBG"))
    dbg = {}
    if KDBG:
        for nm_, shp, dt_ in [
            ("dbg_x0", [128, 2 * SEG], F32),
            ("dbg_k", [128, 2048], F32),
            ("dbg_q2", [128, 2048], F32),
            ("dbg_f", [128, 1152], F32),
            ("dbg_h", [128, L], F32),
            ("dbg_c", [1, L], F32),
            ("dbg_ix", [1, 8], U32),
            ("dbg_wb", [128, 8], F32),
            ("dbg_s", [128, 2 * SEG], F32),
            ("dbg_x1", [128, 2 * SEG], F32),
            ("dbg_xo", [128, 2 * SEG], F32),
        ]:
            dbg[nm_] = nc.declare_dram_parameter(nm_, shp, dt_, isOutput=True)

    fsh = nc.dram_tensor("fsh", [BL * NL, FSH_SZ], F32R)

    with TileContext(nc) as tc:
        ctx = contextlib.ExitStack()
        with ctx:
            wp = ctx.enter_context(tc.tile_pool(name="consts", bufs=1))
            res = ctx.enter_context(tc.tile_pool(name="res", bufs=BL))
            bigq = ctx.enter_context(tc.tile_pool(name="bigq", bufs=2))
            vop = ctx.enter_context(tc.tile_pool(name="vop", bufs=2))
            hp = ctx.enter_context(tc.tile_pool(name="hp", bufs=2))
            fpo = ctx.enter_context(tc.tile_pool(name="fpo", bufs=1))
            pt = ctx.enter_context(tc.tile_pool(name="parts", bufs=5))
            yp = ctx.enter_context(tc.tile_pool(name="yp", bufs=3))
            scr = ctx.enter_context(tc.tile_pool(name="scr", bufs=2))
            tlp = ctx.enter_context(tc.tile_pool(name="tail", bufs=1))
            gp = ctx.enter_context(tc.tile_pool(name="gp", bufs=2))
            spc = ctx.enter_context(tc.tile_pool(name="spc", bufs=1))
            sps = ctx.enter_context(tc.tile_pool(name="sps", bufs=3))
            xep = ctx.enter_context(tc.tile_pool(name="xep", bufs=1))
            ws = ctx.enter_context(tc.tile_pool(name="wstream", bufs=2))
            # PSUM: F 3 banks + FFN accum 2 banks + working 3 banks = 8
            psF = ctx.enter_context(tc.tile_pool(name="psF", bufs=1, space="PSUM"))
            psA = ctx.enter_context(tc.tile_pool(name="psA", bufs=1, space="PSUM"))
            psW = ctx.enter_context(tc.tile_pool(name="psW", bufs=3, space="PSUM"))

            _names = [0]

            def _nm(pfx):
                _names[0] += 1
                return f"{pfx}{_names[0]}"

            # ---- constants to SBUF once
            tokw_sb = wp.tile([63, D], F32R, tag="tokw")
            nc.sync.dma_start(out=tokw_sb[:], in_=tokw[:])
            ones_sb = wp.tile([128, 1], F32R, tag="ones")
            ones2_sb = wp.tile([128, 1], F32, tag="ones2")
            nc.sync.dma_start(out=ones_sb[:], in_=onescol[:])
            nc.sync.dma_start(out=ones2_sb[:], in_=onescolf[:])
            onesr_sb = wp.tile([1, 128], F32, tag="onesr")
            nc.sync.dma_start(out=onesr_sb[:], in_=onesrow[:])
            id_sb = wp.tile([128, 128], F16, tag="id")
            nc.sync.dma_start(out=id_sb[:], in_=identh[:])
            nw_sb = wp.tile([128, 2], F32, tag="nw")  # col t = tile t
            nb_sb = wp.tile([128, 2], F32, tag="nb")
            for t in range(2):
                nc.sync.dma_start(
                    out=nw_sb[:, t : t + 1], in_=nwp[t * 128 : (t + 1) * 128, :]
                )
                nc.sync.dma_start(
                    out=nb_sb[:, t : t + 1], in_=nbp[t * 128 : (t + 1) * 128, :]
                )
            pb_sb = wp.tile([1, 3], F32, tag="pb")
            nc.sync.dma_start(out=pb_sb[:], in_=pb[:])
            neg02 = wp.tile([128, 1], F32, tag="neg02")
            nc.vector.memset(neg02[:], -0.2)
            pw_sb = [None, None]
            for t in range(2):
                pw_sb[t] = wp.tile([128, 3 * L], F16, tag=f"pw{t}", name=f"pw{t}")
                nc.sync.dma_start(
                    out=pw_sb[t][:].rearrange("p (c l) -> p c l", c=3),
                    in_=pw[t * 128 : (t + 1) * 128, :, :],
                )

            # ---- shared per-layer weights (double-buffered across layers)
            def wload(l):
                def tl(name, src, kt, cols, dt=F32R):
                    t_ = ws.tile(
                        [128, cols], dt, tag=f"{name}k{kt}", name=_nm(f"{name}{l}_")
                    )
                    nc.sync.dma_start(
                        out=t_[:], in_=src[l, kt * 128 : (kt + 1) * 128, :]
                    )
                    return t_

                return {
                    "wq": [tl("wq", wq, t, D) for t in range(2)],
                    "wk": [tl("wk", wk, t, D) for t in range(2)],
                    "wvo": [tl("wvo", wvo, t, D) for t in range(2)],
                    "wc1": [tl("wc1", wc1, t, DFF) for t in range(2)],
                    "wc2": [tl("wc2", wc2, t, D, F16) for t in range(8)],
                }

            wsets = {0: wload(0), 1: wload(1)}

            # persistent per-engine delay registers + snapped values
            engs = {
                "ACT": nc.engines[ET.Activation],
                "DVE": nc.engines[ET.DVE],
                "POOL": nc.engines[ET.Pool],
                "PE": nc.engines[ET.PE],
            }
            dreg = {k: e.alloc_register(f"dly_{k}") for k, e in engs.items()}
            dval = {
                k: nc.snap(rg, donate=True, min_val=0, max_val=1023)
                for k, rg in dreg.items()
            }

            # persistent residual tile per batch (updated in place)
            x_sb = {
                b: res.tile([128, 2 * SEG], F32R, tag="res", name=f"x_{b}")
                for b in range(BL)
            }

            def dview(t_):  # [128, 2, 1024] data view of a residual tile
                a = t_[:]
                return bass.AP(
                    a.tensor, a.offset + 2, [list(a.ap[0]), [SEG, 2], [1, 1024]]
                )

            def dseg(t_, seg, c0, n):  # 2D slice of segment data cols
                return t_[:, seg * SEG + 2 + c0 : seg * SEG + 2 + c0 + n]

            def batch_program(b):
                KT = KDBG and b == 0
                xb = x_sb[b]
                # ---- token embedding
                xe_sb = xep.tile([63, L], F32R, tag="xe", name=_nm("xe"))
                nc.sync.dma_start(out=xe_sb[:], in_=xemb[b, :, :])
                for mt in range(2):
                    for ch in range(2):
                        p5 = psW.tile([128, 512], F32, tag="w512", name=_nm("pe_"))
                        nc.tensor.matmul(
                            p5[:],
                            r(tokw_sb[:, mt * 128 : (mt + 1) * 128]),
                            r(xe_sb[:, ch * 512 : (ch + 1) * 512]),
                            start=True,
                            stop=True,
                        )
                        if ch == 0:
                            nc.vector.tensor_copy(dseg(xb, mt, 0, 512), p5[:])
                        else:
                            nc.scalar.copy(dseg(xb, mt, 512, 512), p5[:])
                if KT:
                    nc.sync.dma_start(out=dbg["dbg_x0"][:, :], in_=xb[:].bitcast(F32))
                yield

                def decomp(dst):
                    """dst data = dst data - movmean5(dst data), in place,
                    per segment: fp16 tree (ACT copy + DVE adds), Pool stt."""
                    for sg in range(2):
                        o = sg * SEG
                        nc.vector.tensor_copy(
                            xb[:, o : o + 2],
                            xb[:, o + 2 : o + 3].to_broadcast([128, 2]),
                        )
                        nc.vector.tensor_copy(
                            xb[:, o + 1026 : o + 1028],
                            xb[:, o + 1025 : o + 1026].to_broadcast([128, 2]),
                        )
                    for sg in range(2):
                        o = sg * SEG
                        sh = scr.tile([128, SEG], F16, tag="sh", name=_nm("sh"))
                        nc.scalar.activation(sh[:], dst[:, o : o + SEG], AF.Copy)
                        a2 = scr.tile([128, SEG], F16, tag="a2", name=_nm("a2"))
                        nc.vector.tensor_add(
                            a2[:, 0 : SEG - 1], sh[:, 0 : SEG - 1], sh[:, 1:SEG]
                        )
                        a4 = scr.tile([128, SEG], F16, tag="a4", name=_nm("a4"))
                        nc.vector.tensor_add(
                            a4[:, 0 : SEG - 3], a2[:, 0 : SEG - 3], a2[:, 2 : SEG - 1]
                        )
                        m5 = scr.tile([128, SEG], F16, tag="a2", name=_nm("m5"))
                        nc.vector.tensor_add(
                            m5[:, 0:1024], a4[:, 0:1024], sh[:, 4:SEG]
                        )
                        nc.vector.scalar_tensor_tensor(
                            dst[:, o + 2 : o + 1026],
                            m5[:, 0:1024],
                            -0.2,
                            dst[:, o + 2 : o + 1026],
                            op0=ALU.mult,
                            op1=ALU.add,
                        )

                def qkv_mm(dst_fn, w_l):
                    for mt in range(2):
                        for ch in range(2):
                            p5 = psW.tile(
                                [128, 512], F32, tag="w512", name=_nm("p5")
                            )
                            for kt in range(2):
                                nc.tensor.matmul(
                                    p5[:],
                                    r(w_l[kt][:, mt * 128 : (mt + 1) * 128]),
                                    r(dseg(xb, kt, ch * 512, 512)),
                                    start=(kt == 0),
                                    stop=(kt == 1),
                                )
                            dst_fn(mt, ch, p5)

                for l in range(NL):
                    if b == 0 and l + 1 < NL and (l + 1) not in wsets:
                        wsets[l + 1] = wload(l + 1)
                    wl = wsets[l]
                    unit_last = (b == BL - 1) and (l == NL - 1)
                    tap = KT and l == 0

                    # ---- stage A: q2, k2 (f32r), vo4 (fp16, time-doubled)
                    q2 = bigq.tile([128, 2048], F32R, tag="bigq", name=_nm("q2"))
                    k2 = bigq.tile([128, 2048], F32R, tag="bigq", name=_nm("k2"))
                    vo4 = vop.tile([128, 4096], F16, tag="vo4", name=_nm("vo4"))

                    def q_out(mt, ch, p5):
                        d_ = q2[:, mt * 1024 + ch * 512 : mt * 1024 + (ch + 1) * 512]
                        if ch == 0:
                            nc.vector.tensor_copy(d_, p5[:])
                        else:
                            nc.scalar.copy(d_, p5[:])

                    def k_out(mt, ch, p5):
                        d_ = k2[:, mt * 1024 + ch * 512 : mt * 1024 + (ch + 1) * 512]
                        if ch == 1:
                            nc.vector.tensor_copy(d_, p5[:])
                        else:
                            nc.scalar.copy(d_, p5[:])

                    def v_out(mt, ch, p5):
                        base = mt * 2048 + ch * 512
                        if ch == 0:
                            nc.vector.tensor_copy(vo4[:, base : base + 512], p5[:])
                            nc.scalar.copy(vo4[:, base + 1024 : base + 1536], p5[:])
                        else:
                            nc.scalar.copy(vo4[:, base : base + 512], p5[:])
                            nc.vector.tensor_copy(
                                vo4[:, base + 1024 : base + 1536], p5[:]
                            )

                    qkv_mm(q_out, wl["wq"])
                    qkv_mm(k_out, wl["wk"])
                    qkv_mm(v_out, wl["wvo"])
                    if tap:
                        nc.sync.dma_start(out=dbg["dbg_q2"][:, :], in_=q2[:].bitcast(F32))
                        nc.sync.dma_start(out=dbg["dbg_k"][:, :], in_=k2[:].bitcast(F32))
                    yield

                    # ---- stage B: F[p,u] = sum_i sum_d k[d,128i+p] q[d,(128i+u)%L]
                    fps = psF.tile([128, 1408], F32, tag="F", name=_nm("fps"))
                    for ch in range(3):
                        dst0 = 512 * ch
                        for i in range(8):
                            for kt in range(2):
                                base = 128 * i + 384 * ch
                                st = i == 0 and kt == 0
                                sp_ = i == 7 and kt == 1
                                lhs = k2[
                                    :, kt * 1024 + i * 128 : kt * 1024 + (i + 1) * 128
                                ]
                                if base < 1024 < base + 384:
                                    n1 = 1024 - base
                                    nc.tensor.matmul(
                                        fps[:, dst0 : dst0 + n1],
                                        r(lhs),
                                        r(q2[:, kt * 1024 + base : kt * 1024 + 1024]),
                                        start=st,
                                        stop=False,
                                    )
                                    # start=False: piece 1's start already zeroed
                                    # this bank; start here would wipe piece 1.
                                    nc.tensor.matmul(
                                        fps[:, dst0 + n1 : dst0 + 384],
                                        r(lhs),
                                        r(q2[:, kt * 1024 : kt * 1024 + 384 - n1]),
                                        start=False,
                                        stop=sp_,
                                    )
                                else:
                                    bm = base % 1024
                                    nc.tensor.matmul(
                                        fps[:, dst0 : dst0 + 384],
                                        r(lhs),
                                        r(
                                            q2[
                                                :,
                                                kt * 1024 + bm : kt * 1024 + bm + 384,
                                            ]
                                        ),
                                        start=st,
                                        stop=sp_,
                                    )
                    # copy to SBUF, bounce through DRAM with the shear stride
                    f_sb = fpo.tile([128, 1152], F32R, tag="fsb", name=_nm("fsb"))
                    fa = fps[:]
                    nc.vector.tensor_copy(
                        f_sb[:, 0:768].rearrange("p (c u) -> p c u", c=2),
                        bass.AP(fa.tensor, fa.offset, [list(fa.ap[0]), [512, 2], [1, 384]]),
                    )
                    nc.scalar.copy(f_sb[:, 768:1152], fps[:, 1024:1408])
                    frow = fsh[b * NL + l, :]
                    wview = bass.AP(frow.tensor, frow.offset, [[HW, 128], [1, 1152]])
                    fwr = nc.sync.dma_start(out=wview, in_=f_sb[:, 0:1152])
                    hview = bass.AP(
                        frow.tensor, frow.offset, [[HW + 1, 128], [1, 1024]]
                    )
                    h_sb = hp.tile([128, 1024], F32R, tag="hsb", name=_nm("hsb"))
                    hrd = nc.sync.dma_start(out=h_sb[:], in_=hview)
                    add_dep_helper(
                        hrd.ins, fwr.ins, sync=True, reason="hankel read after write"
                    )
                    if tap:
                        nc.sync.dma_start(
                            out=dbg["dbg_f"][:, :], in_=f_sb[:, 0:1152].bitcast(F32)
                        )
                        nc.sync.dma_start(out=dbg["dbg_h"][:, :], in_=h_sb[:].bitcast(F32))
                    yield

                    # ---- stage C: C[tau] = (1/256) sum_p H[p,tau]; top-6
                    c_sb = spc.tile([1, 1024], F32, tag="csb", name=_nm("cs"))
                    for ch in range(2):
                        cp = psW.tile([1, 512], F32, tag="w512", name=_nm("cp"))
                        nc.tensor.matmul(
                            cp[:],
                            r(ones_sb[:]),
                            r(h_sb[:, ch * 512 : (ch + 1) * 512]),
                            start=True,
                            stop=True,
                        )
                        nc.scalar.activation(
                            c_sb[:1, ch * 512 : (ch + 1) * 512],
                            cp[:],
                            AF.Copy,
                            scale=1.0 / D,
                        )
                    mx = sps.tile([1, 8], F32, tag="mx", name=_nm("mx"))
                    ix = sps.tile([1, 8], U32, tag="ix", name=_nm("ix"))
                    nc.vector.max(out=mx[:], in_=c_sb[:1, 0:1024])
                    nc.vector.max_index(
                        out=ix[:], in_max=mx[:], in_values=c_sb[:1, 0:1024]
                    )
                    negmax = sps.tile([1, 1], F32, tag="negmax", name=_nm("ng"))
                    nc.vector.tensor_scalar_mul(negmax[:], mx[:1, 0:1], -1.0)
                    ex = sps.tile([1, 8], F32, tag="ex", name=_nm("ex"))
                    nc.scalar.activation(
                        ex[:1, 0:TOPK], mx[:1, 0:TOPK], AF.Exp, bias=negmax[:1, 0:1]
                    )
                    esum = sps.tile([1, 1], F32, tag="esum", name=_nm("es"))
                    nc.vector.reduce_sum(esum[:], ex[:1, 0:TOPK], axis=AX.X)
                    rinv = sps.tile([1, 1], F32, tag="rinv", name=_nm("ri"))
                    nc.vector.reciprocal(rinv[:], esum[:])
                    wts = sps.tile([1, 8], F32, tag="wts", name=_nm("wt"))
                    nc.vector.tensor_scalar_mul(
                        wts[:1, 0:TOPK], ex[:1, 0:TOPK], rinv[:1, 0:1]
                    )
                    psw_ = psW.tile([128, TOPK], F32, tag="w512", name=_nm("pw_"))
                    nc.tensor.matmul(
                        psw_[:], onesr_sb[:], wts[:1, 0:TOPK], start=True, stop=True
                    )
                    wb = sps.tile([128, TOPK], F32, tag="wb", name=_nm("wb"))
                    nc.vector.tensor_copy(wb[:], psw_[:])
                    if tap:
                        nc.sync.dma_start(out=dbg["dbg_c"][:, :], in_=c_sb[:1, 0:L])
                        nc.sync.dma_start(out=dbg["dbg_ix"][:, :], in_=ix[:])
                        nc.sync.dma_start(out=dbg["dbg_wb"][:, 0:TOPK], in_=wb[:])
                    yield

                    # ---- stage D: x += sum_i w_i vo[:, (u+d_i)%L]  (in place)
                    v4r = vo4[:].rearrange("p (t u) -> p t u", t=2)

                    def ld(ekey, i):
                        return engs[ekey].reg_load(dreg[ekey], ix[:1, i : i + 1])

                    def p3(t_):
                        return t_[:].rearrange("p (t u) -> p t u", t=2)

                    def dve_init(i, dst3):
                        return nc.vector.tensor_scalar(
                            dst3,
                            v4r[:, :, bass.ds(dval["DVE"], 1024)],
                            wb[:, i : i + 1],
                            None,
                            op0=ALU.mult,
                        )

                    def dve_fma(i, dst3):
                        return nc.vector.scalar_tensor_tensor(
                            dst3,
                            v4r[:, :, bass.ds(dval["DVE"], 1024)],
                            wb[:, i : i + 1],
                            dst3,
                            op0=ALU.mult,
                            op1=ALU.add,
                        )

                    def act_copy(i, dst3):
                        return nc.scalar.activation(
                            dst3,
                            v4r[:, :, bass.ds(dval["ACT"], 1024)],
                            AF.Copy,
                            scale=wb[:, i : i + 1],
                        )

                    def pool_mul(i, dst3):
                        return nc.gpsimd.tensor_mul(
                            dst3,
                            v4r[:, :, bass.ds(dval["POOL"], 1024)],
                            wb[:, i : i + 1].to_broadcast([128, 2, 1024]),
                        )

                    if not unit_last:
                        p0 = pt.tile([128, 2048], F16, tag="part", name=_nm("p0"))
                        p1 = pt.tile([128, 2048], F16, tag="part", name=_nm("p1"))
                        p2 = pt.tile([128, 2048], F16, tag="part", name=_nm("p2"))
                        q1 = pt.tile([128, 2048], F16, tag="part", name=_nm("q1"))
                        q2_ = pt.tile([128, 2048], F16, tag="part", name=_nm("qq"))
                        l0 = ld("DVE", 0)
                        o0 = dve_init(0, p3(p0))
                        dep(o0, l0)
                        l1 = ld("DVE", 1)
                        dep(l1, o0)
                        o1 = dve_fma(1, p3(p0))
                        dep(o1, l1)
                        l2 = ld("ACT", 2)
                        o2 = act_copy(2, p3(p1))
                        dep(o2, l2)
                        l3 = ld("ACT", 3)
                        dep(l3, o2)
                        o3 = act_copy(3, p3(p2))
                        dep(o3, l3)
                        l4 = ld("POOL", 4)
                        o4 = pool_mul(4, p3(q1))
                        dep(o4, l4)
                        l5 = ld("POOL", 5)
                        dep(l5, o4)
                        o5 = pool_mul(5, p3(q2_))
                        dep(o5, l5)
                        nc.gpsimd.tensor_add(q1[:], q1[:], q2_[:])
                        nc.vector.tensor_add(p1[:], p1[:], p2[:])
                        nc.vector.tensor_add(p0[:], p0[:], p1[:])
                        nc.vector.tensor_add(p0[:], p0[:], q1[:])
                        nc.vector.tensor_add(dview(xb), p3(p0), dview(xb))
                    else:
                        p0 = pt.tile([128, 2048], F16, tag="part", name=_nm("p0"))
                        p1 = pt.tile([128, 2048], F16, tag="part", name=_nm("p1"))
                        l0 = ld("DVE", 0)
                        o0 = dve_init(0, p3(p0))
                        dep(o0, l0)
                        l1 = ld("ACT", 1)
                        o1 = act_copy(1, p3(p1))
                        dep(o1, l1)
                        pe = engs["PE"]
                        wds = []
                        for ii, i in enumerate(range(2, 6)):
                            wd = sps.tile(
                                [128, 128], F16, tag=f"wd{ii}", name=_nm("wd")
                            )
                            nc.vector.tensor_scalar(
                                wd[:], id_sb[:], wb[:, i : i + 1], None, op0=ALU.mult
                            )
                            wds.append(wd)
                        pgs = []
                        prev = None
                        for t in range(2):
                            for c in range(2):
                                pg = psW.tile(
                                    [128, 512], F32, tag="w512", name=_nm("pg")
                                )
                                for ii, i in enumerate(range(2, 6)):
                                    lp = pe.reg_load(dreg["PE"], ix[:1, i : i + 1])
                                    if prev is not None:
                                        dep(lp, prev)
                                    al = pe.reg_alu(
                                        dreg["PE"],
                                        dreg["PE"],
                                        2048 * t + 512 * c,
                                        ALU.add,
                                    )
                                    dep(al, lp)
                                    mm = nc.tensor.matmul(
                                        pg[:],
                                        r(wds[ii][:]),
                                        r(vo4[:, bass.ds(dval["PE"], 512)]),
                                        start=(ii == 0),
                                        stop=(ii == 3),
                                    )
                                    dep(mm, al)
                                    prev = mm
                                pgs.append((t, c, pg))
                        nc.vector.tensor_add(p0[:], p0[:], p1[:])
                        nc.vector.tensor_add(dview(xb), p3(p0), dview(xb))
                        for t, c, pg in pgs:
                            nc.vector.tensor_add(
                                dseg(xb, t, c * 512, 512),
                                dseg(xb, t, c * 512, 512),
                                pg[:],
                            )
                    if tap:
                        nc.sync.dma_start(out=dbg["dbg_s"][:, :], in_=xb[:].bitcast(F32))
                    yield

                    # ---- stage E: x1 = decomp(s), in place
                    decomp(xb)
                    if tap:
                        nc.sync.dma_start(out=dbg["dbg_x1"][:, :], in_=xb[:].bitcast(F32))
                    yield

                    # ---- stages F1/F2: FFN halves; y fp16; c2 accum in psum
                    for ch in range(2):
                        acc = psA.tile([128, 1024], F32, tag="ffn", name=_nm("acc"))
                        for ft in range(8):
                            p5 = psW.tile([128, 512], F32, tag="w512", name=_nm("p5"))
                            for kt in range(2):
                                nc.tensor.matmul(
                                    p5[:],
                                    r(wl["wc1"][kt][:, ft * 128 : (ft + 1) * 128]),
                                    r(dseg(xb, kt, ch * 512, 512)),
                                    start=(kt == 0),
                                    stop=(kt == 1),
                                )
                            y_sb = yp.tile([128, 512], F16, tag="y", name=_nm("y"))
                            nc.scalar.activation(y_sb[:], p5[:], AF.Gelu)
                            for mt in range(2):
                                nc.tensor.matmul(
                                    acc[:, mt * 512 : (mt + 1) * 512],
                                    r(wl["wc2"][ft][:, mt * 128 : (mt + 1) * 128]),
                                    r(y_sb[:]),
                                    start=(ft == 0),
                                    stop=(ft == 7),
                                )
                        for mt in range(2):
                            nc.vector.tensor_add(
                                dseg(xb, mt, ch * 512, 512),
                                dseg(xb, mt, ch * 512, 512),
                                acc[:, mt * 512 : (mt + 1) * 512],
                            )
                        yield

                    # ---- stage G: x = decomp(s2), in place
                    decomp(xb)
                    if tap:
                        nc.sync.dma_start(out=dbg["dbg_xo"][:, :], in_=xb[:].bitcast(F32))
                    yield

                # ---- tail: my_layernorm + gelu + head
                xv = [dseg(xb, t, 0, 1024) for t in range(2)]
                xsq = [
                    tlp.tile([128, 1024], F32R, tag=f"xsq{t}", name=_nm("xq"))
                    for t in range(2)
                ]
                for t in range(2):
                    nc.scalar.activation(xsq[t][:], xv[t], AF.Square)
                mu = tlp.tile([1, 1024], F32, tag="mu", name=_nm("mu"))
                ex2 = tlp.tile([1, 1024], F32, tag="ex2", name=_nm("e2"))
                for ch in range(2):
                    cs = psW.tile([1, 512], F32, tag="w512", name=_nm("cs"))
                    for kt in range(2):
                        nc.tensor.matmul(
                            cs[:],
                            r(ones_sb[:]),
                            r(xv[kt][:, ch * 512 : (ch + 1) * 512]),
                            start=(kt == 0),
                            stop=(kt == 1),
                        )
                    nc.scalar.activation(
                        mu[:1, ch * 512 : (ch + 1) * 512], cs[:], AF.Copy, scale=1.0 / D
                    )
                    cq = psW.tile([1, 512], F32, tag="w512", name=_nm("cq"))
                    for kt in range(2):
                        nc.tensor.matmul(
                            cq[:],
                            r(ones_sb[:]),
                            r(xsq[kt][:, ch * 512 : (ch + 1) * 512]),
                            start=(kt == 0),
                            stop=(kt == 1),
                        )
                    nc.scalar.activation(
                        ex2[:1, ch * 512 : (ch + 1) * 512],
                        cq[:],
                        AF.Copy,
                        scale=1.0 / D,
                    )
                epsb = sps.tile([1, 1], F32, tag="epsb", name=_nm("ep"))
                nc.vector.memset(epsb[:], 1e-5)
                musq = spc.tile([1, 1024], F32, tag="csb", name=_nm("mq"))
                nc.vector.tensor_mul(musq[:1, 0:1024], mu[:1, 0:1024], mu[:1, 0:1024])
                nc.vector.tensor_sub(
                    ex2[:1, 0:1024], ex2[:1, 0:1024], musq[:1, 0:1024]
                )
                nc.scalar.activation(
                    ex2[:1, 0:1024], ex2[:1, 0:1024], AF.Sqrt, bias=epsb[:1, 0:1]
                )
                nc.vector.reciprocal(ex2[:1, 0:1024], ex2[:1, 0:1024])  # rstd
                mub = tlp.tile([128, 1024], F32, tag="mub", name=_nm("mb"))
                rstdb = tlp.tile([128, 1024], F32, tag="rstdb", name=_nm("rb"))
                for src, dst in ((mu, mub), (ex2, rstdb)):
                    for ch in range(2):
                        pbd = psW.tile([128, 512], F32, tag="w512", name=_nm("pb_"))
                        nc.tensor.matmul(
                            pbd[:],
                            onesr_sb[:],
                            src[:1, ch * 512 : (ch + 1) * 512],
                            start=True,
                            stop=True,
                        )
                        nc.vector.tensor_copy(dst[:, ch * 512 : (ch + 1) * 512], pbd[:])
                yield
                g_sb = [None, None]
                for t in range(2):
                    xh = xsq[t]  # reuse as xh scratch (f32)
                    nc.vector.tensor_sub(xh[:, 0:L], xv[t], mub[:, 0:L])
                    nc.vector.tensor_mul(xh[:, 0:L], xh[:, 0:L], rstdb[:, 0:L])
                    nc.scalar.activation(
                        xh[:, 0:L],
                        xh[:, 0:L],
                        AF.Identity,
                        bias=nb_sb[:, t : t + 1],
                        scale=nw_sb[:, t : t + 1],
                    )
                    rowm = sps.tile([128, 1], F32, tag="rowm", name=_nm("rm"))
                    nc.vector.reduce_sum(rowm[:], xh[:, 0:L], axis=AX.X)
                    nc.vector.tensor_scalar_mul(rowm[:], rowm[:], 1.0 / L)
                    nc.vector.tensor_scalar_sub(xh[:, 0:L], xh[:, 0:L], rowm[:, 0:1])
                    g_sb[t] = gp.tile([128, 1024], F16, tag="g", name=_nm("g"))
                    nc.scalar.activation(g_sb[t][:, 0:L], xh[:, 0:L], AF.Gelu)

                # head: out[c] = sum_{t,p,l} g[t][p,l] * pw[t][p, c, l] + pb
                hsum = sps.tile([128, 8], F32, tag="hsum", name=_nm("hs"))
                for t in range(2):
                    for c in range(3):
                        hscr = gp.tile([128, 1024], F16, tag="hscr", name=_nm("hc"))
                        nc.vector.tensor_mul(
                            hscr[:, 0:L],
                            g_sb[t][:, 0:L],
                            pw_sb[t][:, c * L : (c + 1) * L],
                        )
                        nc.vector.reduce_sum(
                            hsum[:, t * 3 + c : t * 3 + c + 1], hscr[:, 0:L], axis=AX.X
                        )
                psh = psW.tile([1, 6], F32, tag="w512", name=_nm("ph"))
                nc.tensor.matmul(
                    psh[:], ones2_sb[:], hsum[:, 0:6], start=True, stop=True
                )
                h6 = sps.tile([1, 6], F32, tag="h6", name=_nm("h6"))
                nc.vector.tensor_copy(h6[:], psh[:1, 0:6])
                o3 = sps.tile([1, 3], F32, tag="o3", name=_nm("o3"))
                nc.vector.tensor_add(o3[:], h6[:1, 0:3], h6[:1, 3:6])
                nc.vector.tensor_add(o3[:], o3[:], pb_sb[:])
                nc.sync.dma_start(out=out[b : b + 1, :], in_=o3[:])

            # ---- wavefront driver: batch b leads batch b+1 by STAG stages
            gens = [batch_program(b) for b in range(BL)]
            alive = [True] * BL

            def adv(i, n=1):
                for _ in range(n):
                    if not alive[i]:
                        return
                    try:
                        next(gens[i])
                    except StopIteration:
                        alive[i] = False

            for i in range(BL):
                adv(i, (BL - 1 - i) * STAG)
            while any(alive):
                for i in range(BL):
                    adv(i)

    _split_control_waits(nc)
    return nc


# ---------------------------------------------------------------- host side
_CACHE = {}


def _get_nc():
    if "nc" not in _CACHE:
        _CACHE["nc"] = build_nc()
    return _CACHE["nc"]


def kernel(**inputs):
    x_enc = np.asarray(inputs["x_enc"], dtype=np.float32)  # (B, L, C_IN)
    token_w = np.asarray(inputs["token_w"], dtype=np.float32)
    qw = np.asarray(inputs["qw"], dtype=np.float32)
    kw = np.asarray(inputs["kw"], dtype=np.float32)
    vw = np.asarray(inputs["vw"], dtype=np.float32)
    ow = np.asarray(inputs["ow"], dtype=np.float32)
    c1w = np.asarray(inputs["c1w"], dtype=np.float32)
    c2w = np.asarray(inputs["c2w"], dtype=np.float32)
    norm_w = np.asarray(inputs["norm_w"], dtype=np.float32)
    norm_b = np.asarray(inputs["norm_b"], dtype=np.float32)
    proj_w = np.asarray(inputs["proj_w"], dtype=np.float32)
    proj_b = np.asarray(inputs["proj_b"], dtype=np.float32)

    # host-side layout marshalling (fold ow into vw; no other arithmetic)
    tokw = np.ascontiguousarray(token_w.transpose(1, 2, 0).reshape(63, D))
    xt = x_enc.transpose(0, 2, 1)  # (B, C, L)
    xemb = np.ascontiguousarray(
        np.stack([np.roll(xt, 1 - j, axis=2) for j in range(3)], axis=2).reshape(
            B, 63, L
        )
    )
    # vo = x @ (ow @ vw).T  ->  lhsT[d, m] = (ow @ vw)[m, d].T
    wvo = np.stack([(ow[l] @ vw[l]).T for l in range(NL)])
    shared = {
        "tokw": tokw,
        "wq": np.ascontiguousarray(qw.transpose(0, 2, 1)),
        "wk": np.ascontiguousarray(kw.transpose(0, 2, 1)),
        "wvo": np.ascontiguousarray(wvo),
        "wc1": np.ascontiguousarray(c1w.transpose(0, 2, 1)),
        "wc2": np.ascontiguousarray(c2w.transpose(0, 2, 1)).astype(np.float16),
        "nw": norm_w.reshape(D, 1).copy(),
        "nb": norm_b.reshape(D, 1).copy(),
        "pw": np.ascontiguousarray(
            proj_w.reshape(3, L, D).transpose(2, 0, 1)
        ).astype(np.float16),
        "pb": proj_b.reshape(1, 3).copy(),
        "onescol": np.ones((128, 1), np.float32),
        "onescolf": np.ones((128, 1), np.float32),
        "onesrow": np.ones((1, 128), np.float32),
        "identh": np.eye(128, dtype=np.float16),
    }
    in_maps = []
    for core in range(NCORES):
        m = dict(shared)
        m["xemb"] = np.ascontiguousarray(xemb[core * BL : (core + 1) * BL])
        in_maps.append(m)

    nc = _get_nc()
    res_ = run_bass_kernel_spmd(nc, in_maps, core_ids=list(range(NCORES)))
    out = np.concatenate([res_.results[i]["out"] for i in range(NCORES)], axis=0)
    return out.astype(np.float32)


if __name__ == "__main__":
    import reference

    inputs = reference.setup_inputs()
    got = kernel(**{k: np.asarray(v) for k, v in inputs.items()})
    exp = np.asarray(reference.reference(**inputs))
    rel = np.abs(got - exp).max() / np.abs(exp).max()
    print("Relative error:", rel)


# revision 53
# speedup vs baseline: 1.8192x; 1.1105x over previous
"""Autoformer encoder (B=32, L=1024, D=256, 3 layers) on 8 TRN2 NeuronCores.

Data-parallel over batch (4 batches/core), software-pipelined wavefront:
the 4 batch programs are emitted with a stage stagger (STAG) so matmul
phases of one batch overlap vector phases of another, and layer weights
are loaded once per layer (double-buffered) and shared by all batches.

Precision split (validated against the reference on host):
  - f32r: residual stream x (updated in place through all layers), q2/k,
    the lag-correlation F and C, the decomp outputs. The top-6 lag
    selection is numerically fragile; bf16/fp16 rounding anywhere ahead
    of it flips selections and blows the error to ~3e-2.
  - fp16: vo (= x @ (ow@vw).T, O-projection folded into V on host), the
    gather partial accumulators, the decomp moving-sum tree, FFN y, wc2,
    the classifier head. fp16 gets the DVE 2x/4x fast modes.

AutoCorrelation without FFT: C[tau] = sum_p F[p, p+tau] with
F[p, u] = sum_i sum_d k[d, 128i+p] * q[d, (128i+u) % L] via PSUM-
accumulated matmuls (wrapping chunks split in two, q not duplicated).
The 128-row shear is a DRAM bounce with row stride 1153. Top-6 lags via
vector.max/max_index.

The delay-rolled weighted sum of vo uses register-dynamic slices into a
time-doubled fp16 vo buffer. Each dynamic-AP instruction permanently
consumes ~2 of the executing engine's 49 registers, so the 72 gather
slots are spread: per (batch, layer) unit two DVE, two ACT, two Pool;
the final unit uses ACT 1 / DVE 1 / PE 4 (scaled-identity matmuls with
dynamic rhs).

PSUM: F 3 banks + FFN accumulator 2 banks + 3 working banks = 8.
"""

import contextlib
import numpy as np

import concourse.bass as bass
import concourse.mybir as mybir
from concourse import tile
from concourse.tile import TileContext
from concourse.tile_rust import add_dep_helper
from concourse.vector_clock import ScopedClock
from concourse.bass_utils import run_bass_kernel_spmd

F32 = mybir.dt.float32
F32R = mybir.dt.float32r
F16 = mybir.dt.float16
U32 = mybir.dt.uint32
AF = mybir.ActivationFunctionType
AX = mybir.AxisListType
ALU = mybir.AluOpType
ET = mybir.EngineType

B, L, C_IN = 32, 1024, 21
D, DFF, NL = 256, 1024, 3
TOPK = 6
NCORES = 8
BL = B // NCORES  # batches per core
SEG = 1028  # residual tile segment stride: 2 halo + 1024 + 2 halo

HW = 1153  # F bounce row stride (1152 data + 1 pad)
FSH_SZ = 127 * HW + 1152
STAG = 2  # wavefront stagger in stages between consecutive batches


# ---------------------------------------------------------------- walrus fix
def _patched_drain_and_barrier(self, tick_clock, wait_clock):
    nc = self.nc
    drain_inst = nc.sync.drain()
    wait_clock.add_sem_waits(
        drain_inst.ins, ScopedClock({None: tick_clock.global_clock})
    )
    si = drain_inst.ins.sync_info
    if si is not None and len(si.on_wait) > 1:
        extra = list(si.on_wait[1:])
        del si.on_wait[1:]
        for w in extra:
            n = nc.sync.nop()
            n.ins.sync_info = mybir.SyncInfo(on_update=[], on_wait=[w])
    nc.all_engine_barrier()
    assert self.sems is not None
    popped = nc._tile_sem_poison_stack.pop()
    assert popped is self._sem_poison
    nc.clear_and_free_semaphores(list(self.sems.allocated().values()))
    nc.all_engine_barrier()


tile.TileContext._drain_and_barrier = _patched_drain_and_barrier

_wsctr = [0]


def _split_control_waits(nc):
    """This walrus build allows only ONE sync wait per instruction;
    hoist extras onto NoOps just before, same engine."""
    for fn in nc.m.functions:
        for bb in fn.blocks:
            out = []
            changed = False
            for inst in bb.instructions:
                si = getattr(inst, "sync_info", None)
                if si is not None and len(si.on_wait) > 1:
                    extra = list(si.on_wait[1:])
                    del si.on_wait[1:]
                    for w in extra:
                        _wsctr[0] += 1
                        n = mybir.InstNoOp(
                            name=f"I-waitsplit-{_wsctr[0]}", ins=[], outs=[]
                        )
                        n.engine = inst.engine
                        n.sync_info = mybir.SyncInfo(on_update=[], on_wait=[w])
                        out.append(n)
                        changed = True
                out.append(inst)
            if changed:
                bb.instructions[:] = out


def r(ap):
    return ap


def dep(a, b):
    add_dep_helper(a.ins, b.ins, sync=False, reason="gather order")


# ---------------------------------------------------------------- builder
def build_nc():
    nc = bass.Bass()
    P = lambda name, shape, dt=F32: nc.declare_dram_parameter(
        name, shape, dt, isOutput=False
    )
    xemb = P("xemb", [BL, 63, L], F32R)  # host im2col of token conv input
    tokw = P("tokw", [63, D], F32R)  # lhsT for token conv
    wq = P("wq", [NL, D, D], F32R)  # lhsT (= W.T) per layer
    wk = P("wk", [NL, D, D], F32R)
    wvo = P("wvo", [NL, D, D], F32R)  # lhsT of (ow @ vw)
    wc1 = P("wc1", [NL, D, DFF], F32R)  # lhsT
    wc2 = P("wc2", [NL, DFF, D], F16)  # lhsT, fp16
    nwp = P("nw", [D, 1])
    nbp = P("nb", [D, 1])
    pw = P("pw", [D, 3, L], F16)  # proj_w as [d, class, l]
    pb = P("pb", [1, 3])
    onescol = P("onescol", [128, 1], F32R)
    onescolf = P("onescolf", [128, 1], F32R)
    onesrow = P("onesrow", [1, 128], F32R)
    identh = P("identh", [128, 128], F16)
    identr = P("identr", [128, 128], F32R)
    out = nc.declare_dram_parameter("out", [BL, 3], F32, isOutput=True)
    import os

    KDBG = bool(os.environ.get("KDBG"))
    dbg = {}
    if KDBG:
        for nm_, shp, dt_ in [
            ("dbg_x0", [128, 2 * SEG], F32),
            ("dbg_k", [128, 2048], F32),
            ("dbg_q2", [128, 2048], F32),
            ("dbg_f", [128, 1152], F32),
            ("dbg_h", [128, L], F32),
            ("dbg_c", [1, L], F32),
            ("dbg_ix", [1, 8], U32),
            ("dbg_wb", [128, 8], F32),
            ("dbg_s", [128, 2 * SEG], F32),
            ("dbg_x1", [128, 2 * SEG], F32),
            ("dbg_xo", [128, 2 * SEG], F32),
        ]:
            dbg[nm_] = nc.declare_dram_parameter(nm_, shp, dt_, isOutput=True)

    fsh = nc.dram_tensor("fsh", [BL * NL, FSH_SZ], F32R)

    with TileContext(nc) as tc:
        ctx = contextlib.ExitStack()
        with ctx:
            wp = ctx.enter_context(tc.tile_pool(name="consts", bufs=1))
            res = ctx.enter_context(tc.tile_pool(name="res", bufs=BL))
            bigq = ctx.enter_context(tc.tile_pool(name="bigq", bufs=1))
            vop = ctx.enter_context(tc.tile_pool(name="vop", bufs=2))
            hp = ctx.enter_context(tc.tile_pool(name="hp", bufs=2))
            fpo = ctx.enter_context(tc.tile_pool(name="fpo", bufs=1))
            pt = ctx.enter_context(tc.tile_pool(name="parts", bufs=5))
            yp = ctx.enter_context(tc.tile_pool(name="yp", bufs=2))
            scr = ctx.enter_context(tc.tile_pool(name="scr", bufs=2))
            tlp = ctx.enter_context(tc.tile_pool(name="tail", bufs=1))
            gp = ctx.enter_context(tc.tile_pool(name="gp", bufs=2))
            spc = ctx.enter_context(tc.tile_pool(name="spc", bufs=1))
            sps = ctx.enter_context(tc.tile_pool(name="sps", bufs=2))
            xep = ctx.enter_context(tc.tile_pool(name="xep", bufs=1))
            ws = ctx.enter_context(tc.tile_pool(name="wstream", bufs=2))
            # PSUM: F 3 banks + FFN accum 2 banks + working 3 banks = 8
            psF = ctx.enter_context(tc.tile_pool(name="psF", bufs=1, space="PSUM"))
            psA = ctx.enter_context(tc.tile_pool(name="psA", bufs=1, space="PSUM"))
            psW = ctx.enter_context(tc.tile_pool(name="psW", bufs=3, space="PSUM"))

            _names = [0]

            def _nm(pfx):
                _names[0] += 1
                return f"{pfx}{_names[0]}"

            # ---- constants to SBUF once
            tokw_sb = wp.tile([63, D], F32R, tag="tokw")
            nc.sync.dma_start(out=tokw_sb[:], in_=tokw[:])
            ones_sb = wp.tile([128, 1], F32R, tag="ones")
            ones2_sb = wp.tile([128, 1], F32R, tag="ones2")
            nc.sync.dma_start(out=ones_sb[:], in_=onescol[:])
            nc.sync.dma_start(out=ones2_sb[:], in_=onescolf[:])
            onesr_sb = wp.tile([1, 128], F32R, tag="onesr")
            nc.sync.dma_start(out=onesr_sb[:], in_=onesrow[:])
            id_sb = wp.tile([128, 128], F16, tag="id")
            nc.sync.dma_start(out=id_sb[:], in_=identh[:])
            idr_sb = wp.tile([128, 128], F32R, tag="idr")
            nc.sync.dma_start(out=idr_sb[:], in_=identr[:])
            nw_sb = wp.tile([128, 2], F32, tag="nw")  # col t = tile t
            nb_sb = wp.tile([128, 2], F32, tag="nb")
            for t in range(2):
                nc.sync.dma_start(
                    out=nw_sb[:, t : t + 1], in_=nwp[t * 128 : (t + 1) * 128, :]
                )
                nc.sync.dma_start(
                    out=nb_sb[:, t : t + 1], in_=nbp[t * 128 : (t + 1) * 128, :]
                )
            pb_sb = wp.tile([1, 3], F32, tag="pb")
            nc.sync.dma_start(out=pb_sb[:], in_=pb[:])
            neg02 = wp.tile([128, 1], F32, tag="neg02")
            nc.vector.memset(neg02[:], -0.2)
            pw_sb = [None, None]
            for t in range(2):
                pw_sb[t] = wp.tile([128, 3 * L], F16, tag=f"pw{t}", name=f"pw{t}")
                nc.sync.dma_start(
                    out=pw_sb[t][:].rearrange("p (c l) -> p c l", c=3),
                    in_=pw[t * 128 : (t + 1) * 128, :, :],
                )

            # ---- shared per-layer weights (double-buffered across layers)
            def wload(l):
                def tl(name, src, kt, cols, dt=F32R):
                    t_ = ws.tile(
                        [128, cols], dt, tag=f"{name}k{kt}", name=_nm(f"{name}{l}_")
                    )
                    nc.sync.dma_start(
                        out=t_[:], in_=src[l, kt * 128 : (kt + 1) * 128, :]
                    )
                    return t_

                return {
                    "wq": [tl("wq", wq, t, D) for t in range(2)],
                    "wk": [tl("wk", wk, t, D) for t in range(2)],
                    "wvo": [tl("wvo", wvo, t, D) for t in range(2)],
                    "wc1": [tl("wc1", wc1, t, DFF) for t in range(2)],
                    "wc2": [tl("wc2", wc2, t, D, F16) for t in range(8)],
                }

            wsets = {0: wload(0), 1: wload(1)}

            # persistent per-engine delay registers + snapped values
            engs = {
                "ACT": nc.engines[ET.Activation],
                "DVE": nc.engines[ET.DVE],
                "POOL": nc.engines[ET.Pool],
                "PE": nc.engines[ET.PE],
            }
            dreg = {k: e.alloc_register(f"dly_{k}") for k, e in engs.items()}
            dval = {
                k: nc.snap(rg, donate=True, min_val=0, max_val=1023)
                for k, rg in dreg.items()
            }

            # persistent residual tile per batch (updated in place)
            x_sb = {
                b: res.tile([128, 2 * SEG], F32R, tag="res", name=f"x_{b}")
                for b in range(BL)
            }

            def dview(t_):  # [128, 2, 1024] data view of a residual tile
                a = t_[:]
                return bass.AP(
                    a.tensor, a.offset + 2, [list(a.ap[0]), [SEG, 2], [1, 1024]]
                )

            def dseg(t_, seg, c0, n):  # 2D slice of segment data cols
                return t_[:, seg * SEG + 2 + c0 : seg * SEG + 2 + c0 + n]

            def batch_program(b):
                KT = KDBG and b == 0
                xb = x_sb[b]
                # ---- token embedding
                xe_sb = xep.tile([63, L], F32R, tag="xe", name=_nm("xe"))
                nc.sync.dma_start(out=xe_sb[:], in_=xemb[b, :, :])
                for mt in range(2):
                    for ch in range(2):
                        p5 = psW.tile([128, 512], F32, tag="w512", name=_nm("pe_"))
                        nc.tensor.matmul(
                            p5[:],
                            r(tokw_sb[:, mt * 128 : (mt + 1) * 128]),
                            r(xe_sb[:, ch * 512 : (ch + 1) * 512]),
                            start=True,
                            stop=True,
                        )
                        if ch == 0:
                            nc.vector.tensor_copy(dseg(xb, mt, 0, 512), p5[:])
                        else:
                            nc.scalar.copy(dseg(xb, mt, 512, 512), p5[:])
                if KT:
                    nc.sync.dma_start(out=dbg["dbg_x0"][:, :], in_=xb[:].bitcast(F32))
                yield

                def decomp(dst):
                    """dst data = dst data - movmean5(dst data), in place,
                    per segment: fp16 tree (ACT copy + DVE adds), Pool stt."""
                    for sg in range(2):
                        o = sg * SEG
                        nc.vector.tensor_copy(
                            xb[:, o : o + 2],
                            xb[:, o + 2 : o + 3].to_broadcast([128, 2]),
                        )
                        nc.vector.tensor_copy(
                            xb[:, o + 1026 : o + 1028],
                            xb[:, o + 1025 : o + 1026].to_broadcast([128, 2]),
                        )
                    for sg in range(2):
                        o = sg * SEG
                        sh = scr.tile([128, SEG], F16, tag="sh", name=_nm("sh"))
                        nc.scalar.activation(sh[:], dst[:, o : o + SEG], AF.Copy)
                        a2 = scr.tile([128, SEG], F16, tag="a2", name=_nm("a2"))
                        nc.vector.tensor_add(
                            a2[:, 0 : SEG - 1], sh[:, 0 : SEG - 1], sh[:, 1:SEG]
                        )
                        a4 = scr.tile([128, SEG], F16, tag="a4", name=_nm("a4"))
                        nc.vector.tensor_add(
                            a4[:, 0 : SEG - 3], a2[:, 0 : SEG - 3], a2[:, 2 : SEG - 1]
                        )
                        m5 = scr.tile([128, SEG], F16, tag="a2", name=_nm("m5"))
                        nc.vector.tensor_add(
                            m5[:, 0:1024], a4[:, 0:1024], sh[:, 4:SEG]
                        )
                        nc.vector.scalar_tensor_tensor(
                            dst[:, o + 2 : o + 1026],
                            m5[:, 0:1024],
                            -0.2,
                            dst[:, o + 2 : o + 1026],
                            op0=ALU.mult,
                            op1=ALU.add,
                        )

                def qkv_mm(dst_fn, w_l):
                    for mt in range(2):
                        for ch in range(2):
                            p5 = psW.tile(
                                [128, 512], F32, tag="w512", name=_nm("p5")
                            )
                            for kt in range(2):
                                nc.tensor.matmul(
                                    p5[:],
                                    r(w_l[kt][:, mt * 128 : (mt + 1) * 128]),
                                    r(dseg(xb, kt, ch * 512, 512)),
                                    start=(kt == 0),
                                    stop=(kt == 1),
                                )
                            dst_fn(mt, ch, p5)

                for l in range(NL):
                    if b == 0 and l + 1 < NL and (l + 1) not in wsets:
                        wsets[l + 1] = wload(l + 1)
                    wl = wsets[l]
                    unit_last = (b == BL - 1) and (l == NL - 1)
                    tap = KT and l == 0

                    # ---- stage A: q2 (f32r, +384 wrap cols), k2, vo4 (fp16, doubled)
                    q2 = bigq.tile([128, 2 * 1408], F32R, tag="bigq2", name=_nm("q2"))
                    k2 = bigq.tile([128, 2048], F32R, tag="bigq", name=_nm("k2"))
                    vo4 = vop.tile([128, 4096], F16, tag="vo4", name=_nm("vo4"))

                    def q_out(mt, ch, p5):
                        d_ = q2[:, mt * 1408 + ch * 512 : mt * 1408 + (ch + 1) * 512]
                        if ch == 0:
                            nc.vector.tensor_copy(d_, p5[:])
                            # wrap extension: cols [1024,1408) = q[0:384)
                            nc.scalar.copy(
                                q2[:, mt * 1408 + 1024 : mt * 1408 + 1408],
                                p5[:, 0:384],
                            )
                        else:
                            nc.scalar.copy(d_, p5[:])

                    def k_out(mt, ch, p5):
                        d_ = k2[:, mt * 1024 + ch * 512 : mt * 1024 + (ch + 1) * 512]
                        if ch == 1:
                            nc.vector.tensor_copy(d_, p5[:])
                        else:
                            nc.scalar.copy(d_, p5[:])

                    def v_out(mt, ch, p5):
                        base = mt * 2048 + ch * 512
                        if ch == 0:
                            nc.vector.tensor_copy(vo4[:, base : base + 512], p5[:])
                            nc.scalar.copy(vo4[:, base + 1024 : base + 1536], p5[:])
                        else:
                            nc.scalar.copy(vo4[:, base : base + 512], p5[:])
                            nc.vector.tensor_copy(
                                vo4[:, base + 1024 : base + 1536], p5[:]
                            )

                    qkv_mm(q_out, wl["wq"])
                    qkv_mm(k_out, wl["wk"])
                    qkv_mm(v_out, wl["wvo"])
                    if tap:
                        for sg_ in range(2):
                            nc.sync.dma_start(
                                out=dbg["dbg_q2"][:, sg_ * 1024 : (sg_ + 1) * 1024],
                                in_=q2[:, sg_ * 1408 : sg_ * 1408 + 1024].bitcast(F32),
                            )
                        nc.sync.dma_start(out=dbg["dbg_k"][:, :], in_=k2[:].bitcast(F32))
                    yield

                    # ---- stage B: F[p,u] = sum_i sum_d k[d,128i+p] q[d,(128i+u)%L]
                    fps = psF.tile([128, 1408], F32, tag="F", name=_nm("fps"))
                    for ch in range(3):
                        dst0 = 512 * ch
                        for i in range(8):
                            for kt in range(2):
                                base = 128 * i + 384 * ch
                                bm = base if base < 1024 else base - 1024
                                nc.tensor.matmul(
                                    fps[:, dst0 : dst0 + 384],
                                    r(k2[:, kt * 1024 + i * 128 : kt * 1024 + (i + 1) * 128]),
                                    r(q2[:, kt * 1408 + bm : kt * 1408 + bm + 384]),
                                    start=(i == 0 and kt == 0),
                                    stop=(i == 7 and kt == 1),
                                )
                    # copy to SBUF, bounce through DRAM with the shear stride
                    f_sb = fpo.tile([128, 1152], F32R, tag="fsb", name=_nm("fsb"))
                    fa = fps[:]
                    nc.vector.tensor_copy(
                        f_sb[:, 0:768].rearrange("p (c u) -> p c u", c=2),
                        bass.AP(fa.tensor, fa.offset, [list(fa.ap[0]), [512, 2], [1, 384]]),
                    )
                    nc.scalar.copy(f_sb[:, 768:1152], fps[:, 1024:1408])
                    frow = fsh[b * NL + l, :]
                    wview = bass.AP(frow.tensor, frow.offset, [[HW, 128], [1, 1152]])
                    fwr = nc.sync.dma_start(out=wview, in_=f_sb[:, 0:1152])
                    hview = bass.AP(
                        frow.tensor, frow.offset, [[HW + 1, 128], [1, 1024]]
                    )
                    h_sb = hp.tile([128, 1024], F32R, tag="hsb", name=_nm("hsb"))
                    hrd = nc.sync.dma_start(out=h_sb[:], in_=hview)
                    add_dep_helper(
                        hrd.ins, fwr.ins, sync=True, reason="hankel read after write"
                    )
                    if tap:
                        nc.sync.dma_start(
                            out=dbg["dbg_f"][:, :], in_=f_sb[:, 0:1152].bitcast(F32)
                        )
                        nc.sync.dma_start(out=dbg["dbg_h"][:, :], in_=h_sb[:].bitcast(F32))
                    yield

                    # ---- stage C: C[tau] = (1/256) sum_p H[p,tau]; top-6
                    c_sb = spc.tile([1, 1024], F32, tag="csb", name=_nm("cs"))
                    for ch in range(2):
                        cp = psW.tile([1, 512], F32, tag="w512", name=_nm("cp"))
                        nc.tensor.matmul(
                            cp[:],
                            r(ones_sb[:]),
                            r(h_sb[:, ch * 512 : (ch + 1) * 512]),
                            start=True,
                            stop=True,
                        )
                        nc.scalar.activation(
                            c_sb[:1, ch * 512 : (ch + 1) * 512],
                            cp[:],
                            AF.Copy,
                            scale=1.0 / D,
                        )
                    mx = sps.tile([1, 8], F32, tag="mx", name=_nm("mx"))
                    ix = sps.tile([1, 8], U32, tag="ix", name=_nm("ix"))
                    nc.vector.max(out=mx[:], in_=c_sb[:1, 0:1024])
                    nc.vector.max_index(
                        out=ix[:], in_max=mx[:], in_values=c_sb[:1, 0:1024]
                    )
                    negmax = sps.tile([1, 1], F32, tag="negmax", name=_nm("ng"))
                    nc.vector.tensor_scalar_mul(negmax[:], mx[:1, 0:1], -1.0)
                    ex = sps.tile([1, 8], F32, tag="ex", name=_nm("ex"))
                    nc.scalar.activation(
                        ex[:1, 0:TOPK], mx[:1, 0:TOPK], AF.Exp, bias=negmax[:1, 0:1]
                    )
                    esum = sps.tile([1, 1], F32, tag="esum", name=_nm("es"))
                    nc.vector.reduce_sum(esum[:], ex[:1, 0:TOPK], axis=AX.X)
                    rinv = sps.tile([1, 1], F32, tag="rinv", name=_nm("ri"))
                    nc.vector.reciprocal(rinv[:], esum[:])
                    wts = sps.tile([1, 8], F32R, tag="wts", name=_nm("wt"))
                    nc.vector.tensor_scalar_mul(
                        wts[:1, 0:TOPK], ex[:1, 0:TOPK], rinv[:1, 0:1]
                    )
                    psw_ = psW.tile([128, TOPK], F32, tag="w512", name=_nm("pw_"))
                    nc.tensor.matmul(
                        psw_[:], onesr_sb[:], wts[:1, 0:TOPK], start=True, stop=True
                    )
                    wb = sps.tile([128, TOPK], F32, tag="wb", name=_nm("wb"))
                    nc.vector.tensor_copy(wb[:], psw_[:])
                    if tap:
                        nc.sync.dma_start(out=dbg["dbg_c"][:, :], in_=c_sb[:1, 0:L])
                        nc.sync.dma_start(out=dbg["dbg_ix"][:, :], in_=ix[:])
                        nc.sync.dma_start(out=dbg["dbg_wb"][:, 0:TOPK], in_=wb[:])
                    yield

                    # ---- stage D: x += sum_i w_i vo[:, (u+d_i)%L]  (in place)
                    v4r = vo4[:].rearrange("p (t u) -> p t u", t=2)

                    def ld(ekey, i):
                        return engs[ekey].reg_load(dreg[ekey], ix[:1, i : i + 1])

                    def p3(t_):
                        return t_[:].rearrange("p (t u) -> p t u", t=2)

                    def dve_init(i, dst3):
                        return nc.vector.tensor_scalar(
                            dst3,
                            v4r[:, :, bass.ds(dval["DVE"], 1024)],
                            wb[:, i : i + 1],
                            None,
                            op0=ALU.mult,
                        )

                    def dve_fma(i, dst3):
                        return nc.vector.scalar_tensor_tensor(
                            dst3,
                            v4r[:, :, bass.ds(dval["DVE"], 1024)],
                            wb[:, i : i + 1],
                            dst3,
                            op0=ALU.mult,
                            op1=ALU.add,
                        )

                    def act_copy(i, dst3):
                        return nc.scalar.activation(
                            dst3,
                            v4r[:, :, bass.ds(dval["ACT"], 1024)],
                            AF.Copy,
                            scale=wb[:, i : i + 1],
                        )

                    def pool_mul(i, dst3):
                        return nc.gpsimd.tensor_mul(
                            dst3,
                            v4r[:, :, bass.ds(dval["POOL"], 1024)],
                            wb[:, i : i + 1].to_broadcast([128, 2, 1024]),
                        )

                    if not unit_last:
                        p0 = pt.tile([128, 2048], F16, tag="part", name=_nm("p0"))
                        p1 = pt.tile([128, 2048], F16, tag="part", name=_nm("p1"))
                        p2 = pt.tile([128, 2048], F16, tag="part", name=_nm("p2"))
                        q1 = pt.tile([128, 2048], F16, tag="part", name=_nm("q1"))
                        q2_ = pt.tile([128, 2048], F16, tag="part", name=_nm("qq"))
                        l0 = ld("DVE", 0)
                        o0 = dve_init(0, p3(p0))
                        dep(o0, l0)
                        l1 = ld("DVE", 1)
                        dep(l1, o0)
                        o1 = dve_fma(1, p3(p0))
                        dep(o1, l1)
                        l2 = ld("ACT", 2)
                        o2 = act_copy(2, p3(p1))
                        dep(o2, l2)
                        l3 = ld("ACT", 3)
                        dep(l3, o2)
                        o3 = act_copy(3, p3(p2))
                        dep(o3, l3)
                        l4 = ld("POOL", 4)
                        o4 = pool_mul(4, p3(q1))
                        dep(o4, l4)
                        l5 = ld("POOL", 5)
                        dep(l5, o4)
                        o5 = pool_mul(5, p3(q2_))
                        dep(o5, l5)
                        nc.gpsimd.tensor_add(q1[:], q1[:], q2_[:])
                        nc.vector.tensor_add(p1[:], p1[:], p2[:])
                        nc.vector.tensor_add(p0[:], p0[:], p1[:])
                        nc.vector.tensor_add(p0[:], p0[:], q1[:])
                        nc.vector.tensor_add(dview(xb), p3(p0), dview(xb))
                    else:
                        p0 = pt.tile([128, 2048], F16, tag="part", name=_nm("p0"))
                        p1 = pt.tile([128, 2048], F16, tag="part", name=_nm("p1"))
                        l0 = ld("DVE", 0)
                        o0 = dve_init(0, p3(p0))
                        dep(o0, l0)
                        l1 = ld("ACT", 1)
                        o1 = act_copy(1, p3(p1))
                        dep(o1, l1)
                        pe = engs["PE"]
                        wds = []
                        for ii, i in enumerate(range(2, 6)):
                            wd = sps.tile(
                                [128, 128], F16, tag=f"wd{ii}", name=_nm("wd")
                            )
                            nc.vector.tensor_scalar(
                                wd[:], id_sb[:], wb[:, i : i + 1], None, op0=ALU.mult
                            )
                            wds.append(wd)
                        pgs = []
                        prev = None
                        for t in range(2):
                            for c in range(2):
                                pg = psW.tile(
                                    [128, 512], F32, tag="w512", name=_nm("pg")
                                )
                                for ii, i in enumerate(range(2, 6)):
                                    lp = pe.reg_load(dreg["PE"], ix[:1, i : i + 1])
                                    if prev is not None:
                                        dep(lp, prev)
                                    al = pe.reg_alu(
                                        dreg["PE"],
                                        dreg["PE"],
                                        2048 * t + 512 * c,
                                        ALU.add,
                                    )
                                    dep(al, lp)
                                    mm = nc.tensor.matmul(
                                        pg[:],
                                        r(wds[ii][:]),
                                        r(vo4[:, bass.ds(dval["PE"], 512)]),
                                        start=(ii == 0),
                                        stop=(ii == 3),
                                    )
                                    dep(mm, al)
                                    prev = mm
                                pgs.append((t, c, pg))
                        nc.vector.tensor_add(p0[:], p0[:], p1[:])
                        nc.vector.tensor_add(dview(xb), p3(p0), dview(xb))
                        for t, c, pg in pgs:
                            nc.vector.tensor_add(
                                dseg(xb, t, c * 512, 512),
                                dseg(xb, t, c * 512, 512),
                                pg[:],
                            )
                    if tap:
                        nc.sync.dma_start(out=dbg["dbg_s"][:, :], in_=xb[:].bitcast(F32))
                    yield

                    # ---- stage E: x1 = decomp(s), in place
                    decomp(xb)
                    if tap:
                        nc.sync.dma_start(out=dbg["dbg_x1"][:, :], in_=xb[:].bitcast(F32))
                    yield

                    # ---- stages F1/F2: FFN halves; y fp16; c2 accum in psum
                    for ch in range(2):
                        acc = psA.tile([128, 1024], F32, tag="ffn", name=_nm("acc"))
                        for ft in range(8):
                            p5 = psW.tile([128, 512], F32, tag="w512", name=_nm("p5"))
                            for kt in range(2):
                                nc.tensor.matmul(
                                    p5[:],
                                    r(wl["wc1"][kt][:, ft * 128 : (ft + 1) * 128]),
                                    r(dseg(xb, kt, ch * 512, 512)),
                                    start=(kt == 0),
                                    stop=(kt == 1),
                                )
                            y_sb = yp.tile([128, 512], F16, tag="y", name=_nm("y"))
                            nc.scalar.activation(y_sb[:], p5[:], AF.Gelu)
                            for mt in range(2):
                                nc.tensor.matmul(
                                    acc[:, mt * 512 : (mt + 1) * 512],
                                    r(wl["wc2"][ft][:, mt * 128 : (mt + 1) * 128]),
                                    r(y_sb[:]),
                                    start=(ft == 0),
                                    stop=False,
                                )
                        # fold the residual add into the accumulation group:
                        # acc += I @ x1, then write back via ACT copy
                        for mt in range(2):
                            nc.tensor.matmul(
                                acc[:, mt * 512 : (mt + 1) * 512],
                                r(idr_sb[:]),
                                r(dseg(xb, mt, ch * 512, 512)),
                                start=False,
                                stop=True,
                            )
                        for mt in range(2):
                            nc.scalar.copy(
                                dseg(xb, mt, ch * 512, 512),
                                acc[:, mt * 512 : (mt + 1) * 512],
                            )
                        yield

                    # ---- stage G: x = decomp(s2), in place
                    decomp(xb)
                    if tap:
                        nc.sync.dma_start(out=dbg["dbg_xo"][:, :], in_=xb[:].bitcast(F32))
                    yield

                # ---- tail: my_layernorm + gelu + head
                xv = [dseg(xb, t, 0, 1024) for t in range(2)]
                xsq = [
                    tlp.tile([128, 1024], F32R, tag=f"xsq{t}", name=_nm("xq"))
                    for t in range(2)
                ]
                for t in range(2):
                    nc.scalar.activation(xsq[t][:], xv[t], AF.Square)
                mu = tlp.tile([1, 1024], F32R, tag="mu", name=_nm("mu"))
                ex2 = tlp.tile([1, 1024], F32R, tag="ex2", name=_nm("e2"))
                for ch in range(2):
                    cs = psW.tile([1, 512], F32, tag="w512", name=_nm("cs"))
                    for kt in range(2):
                        nc.tensor.matmul(
                            cs[:],
                            r(ones_sb[:]),
                            r(xv[kt][:, ch * 512 : (ch + 1) * 512]),
                            start=(kt == 0),
                            stop=(kt == 1),
                        )
                    nc.scalar.activation(
                        mu[:1, ch * 512 : (ch + 1) * 512], cs[:], AF.Copy, scale=1.0 / D
                    )
                    cq = psW.tile([1, 512], F32, tag="w512", name=_nm("cq"))
                    for kt in range(2):
                        nc.tensor.matmul(
                            cq[:],
                            r(ones_sb[:]),
                            r(xsq[kt][:, ch * 512 : (ch + 1) * 512]),
                            start=(kt == 0),
                            stop=(kt == 1),
                        )
                    nc.scalar.activation(
                        ex2[:1, ch * 512 : (ch + 1) * 512],
                        cq[:],
                        AF.Copy,
                        scale=1.0 / D,
                    )
                epsb = sps.tile([1, 1], F32, tag="epsb", name=_nm("ep"))
                nc.vector.memset(epsb[:], 1e-5)
                musq = spc.tile([1, 1024], F32, tag="csb", name=_nm("mq"))
                nc.vector.tensor_mul(musq[:1, 0:1024], mu[:1, 0:1024], mu[:1, 0:1024])
                nc.vector.tensor_sub(
                    ex2[:1, 0:1024], ex2[:1, 0:1024], musq[:1, 0:1024]
                )
                nc.scalar.activation(
                    ex2[:1, 0:1024], ex2[:1, 0:1024], AF.Sqrt, bias=epsb[:1, 0:1]
                )
                with nc.allow_low_precision(
                    reason="f32r SBUF storage is bit-identical fp32"
                ):
                    nc.vector.reciprocal(ex2[:1, 0:1024], ex2[:1, 0:1024])  # rstd
                mub = tlp.tile([128, 1024], F32, tag="mub", name=_nm("mb"))
                rstdb = tlp.tile([128, 1024], F32, tag="rstdb", name=_nm("rb"))
                for src, dst in ((mu, mub), (ex2, rstdb)):
                    for ch in range(2):
                        pbd = psW.tile([128, 512], F32, tag="w512", name=_nm("pb_"))
                        nc.tensor.matmul(
                            pbd[:],
                            onesr_sb[:],
                            src[:1, ch * 512 : (ch + 1) * 512],
                            start=True,
                            stop=True,
                        )
                        nc.vector.tensor_copy(dst[:, ch * 512 : (ch + 1) * 512], pbd[:])
                yield
                g_sb = [None, None]
                for t in range(2):
                    xh = xsq[t]  # reuse as xh scratch (f32)
                    nc.vector.tensor_sub(xh[:, 0:L], xv[t], mub[:, 0:L])
                    nc.vector.tensor_mul(xh[:, 0:L], xh[:, 0:L], rstdb[:, 0:L])
                    nc.scalar.activation(
                        xh[:, 0:L],
                        xh[:, 0:L],
                        AF.Identity,
                        bias=nb_sb[:, t : t + 1],
                        scale=nw_sb[:, t : t + 1],
                    )
                    rowm = sps.tile([128, 1], F32, tag="rowm", name=_nm("rm"))
                    nc.vector.reduce_sum(rowm[:], xh[:, 0:L], axis=AX.X)
                    nc.vector.tensor_scalar_mul(rowm[:], rowm[:], 1.0 / L)
                    nc.vector.tensor_scalar_sub(xh[:, 0:L], xh[:, 0:L], rowm[:, 0:1])
                    g_sb[t] = gp.tile([128, 1024], F16, tag="g", name=_nm("g"))
                    nc.scalar.activation(g_sb[t][:, 0:L], xh[:, 0:L], AF.Gelu)

                # head: out[c] = sum_{t,p,l} g[t][p,l] * pw[t][p, c, l] + pb
                hsum = sps.tile([128, 8], F32R, tag="hsum", name=_nm("hs"))
                for t in range(2):
                    for c in range(3):
                        hscr = gp.tile([128, 1024], F16, tag="hscr", name=_nm("hc"))
                        nc.vector.tensor_mul(
                            hscr[:, 0:L],
                            g_sb[t][:, 0:L],
                            pw_sb[t][:, c * L : (c + 1) * L],
                        )
                        with nc.allow_low_precision(
                            reason="f32r SBUF storage is bit-identical fp32"
                        ):
                            nc.vector.reduce_sum(
                                hsum[:, t * 3 + c : t * 3 + c + 1],
                                hscr[:, 0:L],
                                axis=AX.X,
                            )
                psh = psW.tile([1, 6], F32, tag="w512", name=_nm("ph"))
                nc.tensor.matmul(
                    psh[:], ones2_sb[:], hsum[:, 0:6], start=True, stop=True
                )
                h6 = sps.tile([1, 6], F32, tag="h6", name=_nm("h6"))
                nc.vector.tensor_copy(h6[:], psh[:1, 0:6])
                o3 = sps.tile([1, 3], F32, tag="o3", name=_nm("o3"))
                nc.vector.tensor_add(o3[:], h6[:1, 0:3], h6[:1, 3:6])
                nc.vector.tensor_add(o3[:], o3[:], pb_sb[:])
                nc.sync.dma_start(out=out[b : b + 1, :], in_=o3[:])

            # ---- wavefront driver: batch b leads batch b+1 by STAG stages
            gens = [batch_program(b) for b in range(BL)]
            alive = [True] * BL

            def adv(i, n=1):
                for _ in range(n):
                    if not alive[i]:
                        return
                    try:
                        next(gens[i])
                    except StopIteration:
                        alive[i] = False

            for i in range(BL):
                adv(i, (BL - 1 - i) * STAG)
            while any(alive):
                for i in range(BL):
                    adv(i)

    _split_control_waits(nc)
    return nc


# ---------------------------------------------------------------- host side
_CACHE = {}


def _get_nc():
    if "nc" not in _CACHE:
        _CACHE["nc"] = build_nc()
    return _CACHE["nc"]


def kernel(**inputs):
    x_enc = np.asarray(inputs["x_enc"], dtype=np.float32)  # (B, L, C_IN)
    token_w = np.asarray(inputs["token_w"], dtype=np.float32)
    qw = np.asarray(inputs["qw"], dtype=np.float32)
    kw = np.asarray(inputs["kw"], dtype=np.float32)
    vw = np.asarray(inputs["vw"], dtype=np.float32)
    ow = np.asarray(inputs["ow"], dtype=np.float32)
    c1w = np.asarray(inputs["c1w"], dtype=np.float32)
    c2w = np.asarray(inputs["c2w"], dtype=np.float32)
    norm_w = np.asarray(inputs["norm_w"], dtype=np.float32)
    norm_b = np.asarray(inputs["norm_b"], dtype=np.float32)
    proj_w = np.asarray(inputs["proj_w"], dtype=np.float32)
    proj_b = np.asarray(inputs["proj_b"], dtype=np.float32)

    # host-side layout marshalling (fold ow into vw; no other arithmetic)
    tokw = np.ascontiguousarray(token_w.transpose(1, 2, 0).reshape(63, D))
    xt = x_enc.transpose(0, 2, 1)  # (B, C, L)
    xemb = np.ascontiguousarray(
        np.stack([np.roll(xt, 1 - j, axis=2) for j in range(3)], axis=2).reshape(
            B, 63, L
        )
    )
    # vo = x @ (ow @ vw).T  ->  lhsT[d, m] = (ow @ vw)[m, d].T
    wvo = np.stack([(ow[l] @ vw[l]).T for l in range(NL)])
    shared = {
        "tokw": tokw,
        "wq": np.ascontiguousarray(qw.transpose(0, 2, 1)),
        "wk": np.ascontiguousarray(kw.transpose(0, 2, 1)),
        "wvo": np.ascontiguousarray(wvo),
        "wc1": np.ascontiguousarray(c1w.transpose(0, 2, 1)),
        "wc2": np.ascontiguousarray(c2w.transpose(0, 2, 1)).astype(np.float16),
        "nw": norm_w.reshape(D, 1).copy(),
        "nb": norm_b.reshape(D, 1).copy(),
        "pw": np.ascontiguousarray(
            proj_w.reshape(3, L, D).transpose(2, 0, 1)
        ).astype(np.float16),
        "pb": proj_b.reshape(1, 3).copy(),
        "onescol": np.ones((128, 1), np.float32),
        "onescolf": np.ones((128, 1), np.float32),
        "onesrow": np.ones((1, 128), np.float32),
        "identh": np.eye(128, dtype=np.float16),
        "identr": np.eye(128, dtype=np.float32),
    }
    in_maps = []
    for core in range(NCORES):
        m = dict(shared)
        m["xemb"] = np.ascontiguousarray(xemb[core * BL : (core + 1) * BL])
        in_maps.append(m)

    nc = _get_nc()
    res_ = run_bass_kernel_spmd(nc, in_maps, core_ids=list(range(NCORES)))
    out = np.concatenate([res_.results[i]["out"] for i in range(NCORES)], axis=0)
    return out.astype(np.float32)


if __name__ == "__main__":
    import reference

    inputs = reference.setup_inputs()
    got = kernel(**{k: np.asarray(v) for k, v in inputs.items()})
    exp = np.asarray(reference.reference(**inputs))
    rel = np.abs(got - exp).max() / np.abs(exp).max()
    print("Relative error:", rel)
